# revision 26
# baseline (speedup 1.0000x reference)
"""Self-contained Trainium2 Bass kernel for the Flux-style DoubleStreamBlock.

Strategy (8 NeuronCores, SPMD via run_bass_kernel_spmd):
  Launch A: modulation GEMV for the (sh1, sc1) chunks, sharded over output
            rows, + LayerNorm-1 token statistics (token-sharded).
  Launch B: modulation GEMV for the remaining chunks, then head-parallel
            (2 heads/core) QKV + QKNorm + RoPE + attention over the full KV
            cache + row-parallel proj partial sums.
            - LayerNorm-1 + modulation are folded into the QKV matmul:
              qkv[o,t] = rstd[t]*(W'^T x)[o,t] - (mu*rstd)[t]*s[o] + d[o]+b[o]
              with W' = W*(1+sc1)[k], s = col-sums of W', d = W^T sh1.
              So x feeds the PE straight from DMA with no vector-engine prep.
            - The cache scatter is eliminated by reordering keys: softmax is
              permutation-invariant over key positions, so each core gets the
              768 "dead" cache rows (host-gathered) plus the 2304 fresh k/v.
  Launch C: token-sharded reduction of proj partials + gate/residual + LN2.
  Launch D: MLP with the 8192-dim hidden sharded 8x (Megatron-style).
  Launch E: token-sharded reduction of MLP partials + gate/residual = output.

The host only slices / transposes / concatenates / broadcasts numpy arrays
between launches (plus fp32->fp32r / fp32->bf16 format casts); every
arithmetic op of the reference runs on-device.

dtype scheme: moving (rhs) matmul operands are float32r (TF32-like, full PE
rate at moving-dim >= 256, pre-rounded or written via an f32r-typed AP so
the walrus fp32r verifier is satisfied). Stationary weight operands are
bf16 (fast weight load, half the DMA). Attention k/v/q stay f32r for
precision. Cross-core partial sums travel as bf16.
"""

import numpy as np
import ml_dtypes

import concourse.bacc as bacc
import concourse.mybir as mybir
import concourse.tile as tile
from concourse import bass_utils
from neuron_dtypes import static_cast_fp32_to_fp32r

dt = mybir.dt
AF = mybir.ActivationFunctionType
ALU = mybir.AluOpType
F32 = dt.float32
F32R = dt.float32r
BF16 = dt.bfloat16

# Problem shapes (hardcoded per contract).
L_TXT, L_IMG = 256, 2048
L_LIVE, L_FULL = 2304, 3072
HID, H, D, MLP = 2048, 16, 128, 8192
EPS = 1e-6
NCORES = 8
HPC = H // NCORES            # heads per core = 2
L_DEAD = L_FULL - L_LIVE     # 768
KC = HID // 128              # 16 hidden-dim chunks
MC = MLP // NCORES // 128    # 8 mlp-hidden chunks per core
SC = L_FULL // 128           # 24 key-position chunks
INV_SQRT_D = float(1.0 / np.sqrt(D))

# token chunks (stream-aligned: txt tokens first, then img)
TCH = [(0, 256, 't'), (256, 768, 'i'), (768, 1280, 'i'),
       (1280, 1792, 'i'), (1792, 2304, 'i')]

# ownership split for token-sharded launches C/E: core c owns
# txt[c*32:(c+1)*32] + img[c*256:(c+1)*256]  -> uniform 32/256 col split.
TOK_T, TOK_I = L_TXT // NCORES, L_IMG // NCORES   # 32, 256
TOK = TOK_T + TOK_I                               # 288


def _new_nc():
    return bacc.Bacc("TRN2", target_bir_lowering=False, debug=False,
                     enable_asserts=True, num_devices=NCORES)


def _f32(ap):
    return ap.bitcast(F32)


def _emit_rsqrt(nc, pool, out, in_ap, scale, bias, name):
    """out = 1/sqrt(in*scale + bias), via ACT Sqrt + DVE reciprocal + one
    Newton polish (ACT Sqrt's spline budget is loose)."""
    p, f = out.partition_size(), out.free_size()
    v = pool.tile([p, f], F32, name=f"{name}_v")
    s = pool.tile([p, f], F32, name=f"{name}_s")
    t = pool.tile([p, f], F32, name=f"{name}_t")
    b = pool.tile([p, 1], F32, name=f"{name}_b")
    nc.vector.memset(b[:, :], bias)
    # v = in*scale + bias ; s = sqrt(v)
    nc.scalar.activation(v[:, :], in_ap, AF.Identity, bias=b[:, 0:1], scale=scale)
    nc.scalar.activation(s[:, :], in_ap, AF.Sqrt, bias=b[:, 0:1], scale=scale)
    nc.vector.reciprocal(out, s[:, :])                      # y ~ rsqrt(v)
    # Newton: y' = y*(1.5 - 0.5*v*y^2)
    nc.vector.tensor_tensor(t[:, :], out, out, ALU.mult)    # y^2
    nc.vector.tensor_tensor(t[:, :], t[:, :], v[:, :], ALU.mult)
    nc.vector.tensor_scalar(t[:, :], t[:, :], -0.5, 1.5, ALU.mult, ALU.add)
    nc.vector.tensor_tensor(out, out, t[:, :], ALU.mult)


def _emit_mod_gemv(nc, pools, silu, modw_ap, modb_ap, ncols, out_ap):
    """out[1, ncols] = silu^T @ modw + modb   (modw bf16 [HID, ncols])."""
    sp, wp, pp = pools
    btile = sp.tile([1, ncols], F32, name="modbt")
    nc.sync.dma_start(btile[:, :], modb_ap[:, :])
    nch = ncols // 512
    psums = [pp.tile([1, 512], F32, name=f"gmm{o}") for o in range(nch)]
    for k in range(KC):
        wt = wp.tile([128, ncols], BF16, name="gwt")
        nc.sync.dma_start(wt[:, :], modw_ap[k * 128:(k + 1) * 128, :])
        for o in range(nch):
            nc.tensor.matmul(psums[o][:, :], silu[:, k:k + 1],
                             wt[:, o * 512:(o + 1) * 512],
                             start=(k == 0), stop=(k == KC - 1))
    mout = sp.tile([1, ncols], F32, name="gmout")
    for o in range(nch):
        nc.vector.tensor_tensor(mout[:, o * 512:(o + 1) * 512], psums[o][:, :],
                                btile[:, o * 512:(o + 1) * 512], ALU.add)
    nc.sync.dma_start(out_ap[:, :], mout[:, :])


# --------------------------------------------------------------------------
# Launch A: modulation GEMV for sh1/sc1 (row-sharded) + LN1 stats
# --------------------------------------------------------------------------
def build_A():
    nc = _new_nc()
    vec_t = nc.dram_tensor("vec_t", [HID, 1], F32, kind="ExternalInput").ap()
    modw = nc.dram_tensor("modw", [HID, 1024], BF16, kind="ExternalInput").ap()
    modb = nc.dram_tensor("modb", [1, 1024], F32, kind="ExternalInput").ap()
    statx = nc.dram_tensor("statx", [512, HID], BF16, kind="ExternalInput").ap()
    modo = nc.dram_tensor("modo", [1, 1024], F32, kind="ExternalOutput").ap()
    stato = nc.dram_tensor("stato", [2, 512], F32, kind="ExternalOutput").ap()

    with tile.TileContext(nc) as tc:
        with tc.tile_pool(name="small", bufs=1) as sp, \
             tc.tile_pool(name="wts", bufs=3) as wp, \
             tc.tile_pool(name="stat", bufs=2) as st, \
             tc.tile_pool(name="ps", bufs=1, space="PSUM") as pp:
            vraw = sp.tile([128, KC], F32)
            nc.sync.dma_start(vraw[:, :], vec_t.rearrange("(c p) x -> p (c x)", p=128))
            silu = sp.tile([128, KC], BF16)
            nc.scalar.activation(silu[:, :], vraw[:, :], AF.Silu)
            _emit_mod_gemv(nc, (sp, wp, pp), silu, modw, modb, 1024, modo)

            # ---- LN1 stats on 512 tokens (token-major, batched rsqrt) ----
            mu4 = sp.tile([128, 4], F32)
            var4 = sp.tile([128, 4], F32)
            for i in range(4):
                xt = st.tile([128, HID], BF16, name="xt")
                nc.sync.dma_start(xt[:, :], statx[i * 128:(i + 1) * 128, :])
                sq = st.tile([128, HID], F32, name="sq")
                nc.scalar.activation(sq[:, :], xt[:, :], AF.Square)
                s1 = st.tile([128, 1], F32, name="s1")
                s2 = st.tile([128, 1], F32, name="s2")
                nc.vector.tensor_reduce(s1[:, :], xt[:, :], mybir.AxisListType.X, ALU.add)
                nc.vector.tensor_reduce(s2[:, :], sq[:, :], mybir.AxisListType.X, ALU.add)
                nc.vector.tensor_scalar(mu4[:, i:i + 1], s1[:, :], 1.0 / HID, None, ALU.mult)
                nc.vector.tensor_scalar(var4[:, i:i + 1], s2[:, :], 1.0 / HID, None, ALU.mult)
            mu2 = sp.tile([128, 4], F32)
            nc.vector.tensor_tensor(mu2[:, :], mu4[:, :], mu4[:, :], ALU.mult)
            nc.vector.tensor_tensor(var4[:, :], var4[:, :], mu2[:, :], ALU.subtract)
            rstd4 = sp.tile([128, 4], F32)
            _emit_rsqrt(nc, sp, rstd4[:, :], var4[:, :], 1.0, EPS, "rs")
            murstd4 = sp.tile([128, 4], F32)
            nc.vector.tensor_tensor(murstd4[:, :], mu4[:, :], rstd4[:, :], ALU.mult)
            for i in range(4):
                nc.sync.dma_start(stato[0:1, i * 128:(i + 1) * 128], rstd4[:, i:i + 1])
                nc.sync.dma_start(stato[1:2, i * 128:(i + 1) * 128], murstd4[:, i:i + 1])
    nc.compile()
    return nc


# --------------------------------------------------------------------------
# Launch B: mod2 GEMV + QKV(+LN1 fold) + QKNorm + RoPE + attention + proj
# --------------------------------------------------------------------------
def build_B():
    nc = _new_nc()
    inp = {}

    def din(name, shape, d=F32):
        inp[name] = nc.dram_tensor(name, shape, d, kind="ExternalInput").ap()

    din("vec_t", [HID, 1])
    din("modw2", [HID, 2048], BF16)
    din("modb2", [1, 2048])
    din("xt", [HID, L_LIVE], BF16)
    din("qkvw_t", [HID, 6 * 128], BF16); din("qkvw_i", [HID, 6 * 128], BF16)
    din("qkvb_t", [128, 6]); din("qkvb_i", [128, 6])
    din("projw_t", [HPC * 128, HID], BF16); din("projw_i", [HPC * 128, HID], BF16)
    din("sc1_t", [128, KC]); din("sh1_t", [128, KC], BF16)
    din("sc1_i", [128, KC]); din("sh1_i", [128, KC], BF16)
    din("arep", [128, L_LIVE]); din("brep", [128, L_LIVE])
    din("pea", [128, L_LIVE]); din("peb", [128, L_LIVE])
    for s in "ti":
        for m in ("sevq", "sodq", "sevk", "sodk"):
            din(f"{m}_{s}", [128, 128], F32R)
    din("ident", [128, 128])
    din("onesc", [128, 1], F32R)
    din("onescb", [128, 1], BF16)
    din("onesr", [128, 128], F32R)
    din("kdead", [HPC * 128, L_DEAD], F32R)
    din("vdead", [L_DEAD, HPC * 128], BF16)
    pp_out = nc.dram_tensor("pp", [HID, L_LIVE], BF16, kind="ExternalOutput").ap()
    modo2 = nc.dram_tensor("modo2", [1, 2048], F32, kind="ExternalOutput").ap()

    with tile.TileContext(nc) as tc:
        with tc.tile_pool(name="const", bufs=1) as cp, \
             tc.tile_pool(name="state", bufs=1) as stp:
            ones_col = cp.tile([128, 1], F32R)
            nc.sync.dma_start(ones_col[:, :], inp["onesc"][:, :])
            ones_colb = cp.tile([128, 1], BF16)
            nc.sync.dma_start(ones_colb[:, :], inp["onescb"][:, :])
            ones_row = cp.tile([128, 128], F32R)
            nc.sync.dma_start(ones_row[:, :], inp["onesr"][:, :])
            ident = cp.tile([128, 128], F32)
            nc.sync.dma_start(ident[:, :], inp["ident"][:, :])
            smat = {}
            for s in "ti":
                for m in ("sevq", "sodq", "sevk", "sodk"):
                    t_ = cp.tile([128, 128], F32R, name=f"{m}{s}")
                    nc.sync.dma_start(t_[:, :], inp[f"{m}_{s}"][:, :])
                    smat[(m, s)] = t_
            c1 = {}; sh1 = {}; qkvb = {}
            for s in "ti":
                c = cp.tile([128, KC], F32, name=f"c1{s}")
                nc.sync.dma_start(c[:, :], inp[f"sc1_{s}"][:, :])
                nc.vector.tensor_scalar(c[:, :], c[:, :], 1.0, None, ALU.add)
                c1[s] = c
                hh = cp.tile([128, KC], BF16, name=f"sh1{s}")
                nc.sync.dma_start(hh[:, :], inp[f"sh1_{s}"][:, :])
                sh1[s] = hh
                bb = cp.tile([128, 6], F32, name=f"qb{s}")
                nc.sync.dma_start(bb[:, :], inp[f"qkvb_{s}"][:, :])
                qkvb[s] = bb

            # persistent attention state (per head)
            kall = [stp.tile([128, L_FULL], F32R, name=f"kall{h}") for h in range(HPC)]
            vfull = [stp.tile([128, L_FULL], BF16, name=f"vfull{h}") for h in range(HPC)]
            qrot = [stp.tile([128, L_LIVE], F32R, name=f"qrot{h}") for h in range(HPC)]
            for h in range(HPC):
                nc.sync.dma_start(kall[h][:, 0:L_DEAD],
                                  inp["kdead"][h * 128:(h + 1) * 128, :])
                for c6 in range(L_DEAD // 128):
                    nc.sync.dma_start(
                        vfull[h][:, c6 * 128:(c6 + 1) * 128],
                        inp["vdead"][c6 * 128:(c6 + 1) * 128, h * 128:(h + 1) * 128])

            # ---------------- B1: QKV (LN folded) + qknorm + rope ----------
            for s, chunks in (('t', TCH[:1]), ('i', TCH[1:])):
                with tc.tile_pool(name=f"w{s}", bufs=1) as wpool, \
                     tc.tile_pool(name=f"b1{s}", bufs=2) as b1p, \
                     tc.tile_pool(name=f"b1s{s}", bufs=1) as b1small:
                    wq = []
                    for k in range(KC):
                        w_ = wpool.tile([128, 6 * 128], BF16, name=f"wq{k}")
                        nc.sync.dma_start(w_[:, :], inp[f"qkvw_{s}"][k * 128:(k + 1) * 128, :])
                        wq.append(w_)
                    # d[o] = W^T sh1 (before W' overwrite); then W'=W*(1+sc1);
                    # s[o] = col-sums of W'; svec = -s; bias2_j = d + qkv_b
                    svec = b1small.tile([128, 6], F32, name="svec")
                    bias2 = b1small.tile([128, 6], F32, name="bias2")
                    with tc.tile_pool(name=f"psd{s}", bufs=2, space="PSUM") as psd:
                        for j in range(6):
                            dp = psd.tile([128, 1], F32, name="dp")
                            for k in range(KC):
                                nc.tensor.matmul(dp[:, :],
                                                 wq[k][:, j * 128:(j + 1) * 128],
                                                 sh1[s][:, k:k + 1],
                                                 start=(k == 0), stop=(k == KC - 1))
                            nc.vector.tensor_tensor(bias2[:, j:j + 1], dp[:, :],
                                                    qkvb[s][:, j:j + 1], ALU.add)
                        # W' = W * (1+sc1)[k]  (in place)
                        for k in range(KC):
                            nc.vector.tensor_scalar(wq[k][:, :], wq[k][:, :],
                                                    c1[s][:, k:k + 1], None, ALU.mult)
                        for j in range(6):
                            sp_ = psd.tile([128, 1], F32, name="sp_")
                            for k in range(KC):
                                nc.tensor.matmul(sp_[:, :],
                                                 wq[k][:, j * 128:(j + 1) * 128],
                                                 ones_colb[:, :],
                                                 start=(k == 0), stop=(k == KC - 1))
                            nc.vector.tensor_scalar(svec[:, j:j + 1], sp_[:, :],
                                                    -1.0, None, ALU.mult)
                    with tc.tile_pool(name=f"psq{s}", bufs=2, space="PSUM") as psq, \
                         tc.tile_pool(name=f"pse{s}", bufs=1, space="PSUM") as pse, \
                         tc.tile_pool(name=f"psx{s}", bufs=1, space="PSUM") as psx:
                      xt3 = inp["xt"].rearrange("(k p) t -> p k t", p=128)

                      def emit_qkv(t0, t1):
                        tn = t1 - t0
                        xfat = b1p.tile([128, KC * tn], BF16, name="xfat")
                        nc.sync.dma_start(xfat[:, :], xt3[:, :, t0:t1])
                        arep_t = b1small.tile([128, tn], F32, name="arep_t", bufs=2)
                        brep_t = b1small.tile([128, tn], F32, name="brep_t", bufs=2)
                        pea_t = b1small.tile([128, tn], F32, name="pea_t", bufs=2)
                        peb_t = b1small.tile([128, tn], F32, name="peb_t", bufs=2)
                        nc.sync.dma_start(arep_t[:, :], inp["arep"][:, t0:t1])
                        nc.sync.dma_start(brep_t[:, :], inp["brep"][:, t0:t1])
                        nc.sync.dma_start(pea_t[:, :], inp["pea"][:, t0:t1])
                        nc.sync.dma_start(peb_t[:, :], inp["peb"][:, t0:t1])
                        # QKV: G = W'^T x ; out = a[t]*G + (-s[o]*b[t] + bias2[o])
                        raw = []
                        for j in range(6):
                            pq = psq.tile([128, tn], F32, name="pq")
                            for k in range(KC):
                                nc.tensor.matmul(
                                    pq[:, :], wq[k][:, j * 128:(j + 1) * 128],
                                    xfat[:, k * tn:(k + 1) * tn],
                                    start=(k == 0), stop=(k == KC - 1))
                            r = b1p.tile([128, tn], F32R if j < 4 else F32,
                                         name=f"raw{j}", bufs=2)
                            r1 = b1small.tile([128, tn], F32, name="r1")
                            nc.vector.tensor_scalar(r1[:, :], brep_t[:, :],
                                                    svec[:, j:j + 1],
                                                    bias2[:, j:j + 1],
                                                    ALU.mult, ALU.add)
                            e1 = b1small.tile([128, tn], F32, name="e1")
                            nc.vector.tensor_tensor(e1[:, :], pq[:, :],
                                                    arep_t[:, :], ALU.mult)
                            nc.vector.tensor_tensor(r[:, :], e1[:, :], r1[:, :],
                                                    ALU.add)
                            raw.append(r)
                        return (t0, t1, raw, pea_t, peb_t)

                      def emit_tail(t0, t1, raw, pea_t, peb_t):
                        tn = t1 - t0
                        # v: transpose into vfull rows
                        for h in range(HPC):
                            for j1 in range(tn // 128):
                                tp = psx.tile([128, 128], F32, name="tp")
                                nc.tensor.transpose(tp[:, :],
                                                    raw[4 + h][:, j1 * 128:(j1 + 1) * 128],
                                                    ident[:, :])
                                col0 = L_DEAD + t0 + j1 * 128
                                nc.scalar.activation(vfull[h][:, col0:col0 + 128],
                                                     tp[:, :], AF.Copy)
                        # q/k: rmsnorm stats (batched over the 4 q/k tiles)
                        QK = [(h, w) for h in range(HPC) for w in ('q', 'k')]
                        rs4 = b1small.tile([128, tn], F32, name="rs4", bufs=2)
                        for r, (h, w) in enumerate(QK):
                            src_ = raw[h] if w == 'q' else raw[2 + h]
                            sqt = b1small.tile([128, tn], F32R, name="sqt", bufs=2)
                            nc.scalar.activation(sqt[:, :], _f32(src_[:, :]), AF.Square)
                            rsum = psx.tile([1, tn], F32, name="rsum", bufs=2)
                            nc.tensor.matmul(rsum[:, :], ones_col[:, :], sqt[:, :])
                            nc.scalar.activation(rs4[32 * r:32 * r + 1, :], rsum[:, :], AF.Copy)
                        rr4 = b1small.tile([128, tn], F32, name="rr4", bufs=2)
                        _emit_rsqrt(nc, b1small, rr4[:, :], rs4[:, :],
                                    1.0 / D, EPS, "rr")
                        rr4_r = b1small.tile([128, tn], F32R, name="rr4_r", bufs=2)
                        nc.vector.tensor_copy(rr4_r[:, :], rr4[:, :])
                        # rope: dst = (PEa*(Sev q) + PEb*(Sod q)) * rrms
                        for r, (h, w) in enumerate(QK):
                            mev, mod_ = ("sevq", "sodq") if w == 'q' else ("sevk", "sodk")
                            src_ = raw[h] if w == 'q' else raw[2 + h]
                            rrb = psx.tile([128, tn], F32, name="rrb", bufs=1)
                            nc.tensor.matmul(rrb[:, :], ones_row[32 * r:32 * r + 1, :],
                                             rr4_r[32 * r:32 * r + 1, :],
                                             tile_position=(32 * r, 0))
                            ev = pse.tile([128, tn], F32, name="ev")
                            od = pse.tile([128, tn], F32, name="od")
                            nc.tensor.matmul(ev[:, :], smat[(mev, s)][:, :], src_[:, :])
                            nc.tensor.matmul(od[:, :], smat[(mod_, s)][:, :], src_[:, :])
                            t1_ = b1small.tile([128, tn], F32, name="t1_", bufs=2)
                            t2_ = b1small.tile([128, tn], F32, name="t2_", bufs=2)
                            nc.vector.tensor_tensor(t1_[:, :], ev[:, :], pea_t[:, :], ALU.mult)
                            nc.vector.tensor_tensor(t2_[:, :], od[:, :], peb_t[:, :], ALU.mult)
                            nc.vector.tensor_tensor(t1_[:, :], t1_[:, :], t2_[:, :], ALU.add)
                            if w == 'q':
                                dst = qrot[h][:, t0:t1]
                            else:
                                dst = kall[h][:, L_DEAD + t0:L_DEAD + t1]
                            nc.vector.tensor_tensor(dst, t1_[:, :], rrb[:, :], ALU.mult)

                      pend = None
                      for (t0, t1, _s) in chunks:
                        st_ = emit_qkv(t0, t1)
                        if pend is not None:
                            emit_tail(*pend)
                        pend = st_
                      emit_tail(*pend)

            # ---- mod2 GEMV (PE covers B1->B2 transition) ----
            with tc.tile_pool(name="gsm", bufs=1) as gsp, \
                 tc.tile_pool(name="gw", bufs=3) as gwp, \
                 tc.tile_pool(name="gps", bufs=1, space="PSUM") as gpp:
                vraw = gsp.tile([128, KC], F32)
                nc.sync.dma_start(vraw[:, :],
                                  inp["vec_t"].rearrange("(c p) x -> p (c x)", p=128))
                silu = gsp.tile([128, KC], BF16)
                nc.scalar.activation(silu[:, :], vraw[:, :], AF.Silu)
                _emit_mod_gemv(nc, (gsp, gwp, gpp), silu, inp["modw2"],
                               inp["modb2"], 2048, modo2)

            # ---------------- B2+B3: attention + proj partial ----------------
            with tc.tile_pool(name="projw", bufs=1) as pwp, \
                 tc.tile_pool(name="pfat", bufs=2) as pfp, \
                 tc.tile_pool(name="attnp", bufs=2) as atp, \
                 tc.tile_pool(name="b2small", bufs=2) as b2s, \
                 tc.tile_pool(name="pss", bufs=2, space="PSUM") as pss, \
                 tc.tile_pool(name="psa", bufs=1, space="PSUM") as psa, \
                 tc.tile_pool(name="psp", bufs=1, space="PSUM") as psp:
                pw = {}
                for s in "ti":
                    for hc in range(HPC):
                        w_ = pwp.tile([128, HID], BF16, name=f"pw{s}{hc}")
                        nc.sync.dma_start(w_[:, :],
                                          inp[f"projw_{s}"][hc * 128:(hc + 1) * 128, :])
                        pw[(s, hc)] = w_
                pp3 = pp_out.rearrange("(o p) t -> p o t", p=128)
                attn_by_chunk = {}

                def emit_scores(u):
                    (t0, t1, s, h) = u
                    tn = t1 - t0
                    pfat = pfp.tile([128, SC * tn], BF16, name="pfat")
                    for sc2 in range(SC // 2):
                        ps_ = pss.tile([128, 2 * tn], F32, name="ps_")
                        for half in range(2):
                            sc = 2 * sc2 + half
                            nc.tensor.matmul(ps_[:, half * tn:(half + 1) * tn],
                                             kall[h][:, sc * 128:(sc + 1) * 128],
                                             qrot[h][:, t0:t1])
                        nc.scalar.activation(pfat[:, 2 * sc2 * tn:(2 * sc2 + 2) * tn],
                                             ps_[:, :], AF.Exp, scale=INV_SQRT_D)
                    return pfat

                def emit_tail(u, pfat):
                    (t0, t1, s, h) = u
                    tn = t1 - t0
                    rs = psa.tile([1, tn], F32, name="rs")
                    for sc in range(SC):
                        nc.tensor.matmul(rs[:, :], ones_colb[:, :],
                                         pfat[:, sc * tn:(sc + 1) * tn],
                                         start=(sc == 0), stop=(sc == SC - 1))
                    av = psa.tile([128, tn], F32, name="av")
                    for sc in range(SC):
                        nc.tensor.matmul(av[:, :],
                                         vfull[h][:, sc * 128:(sc + 1) * 128],
                                         pfat[:, sc * tn:(sc + 1) * tn],
                                         start=(sc == 0), stop=(sc == SC - 1))
                    rcp = b2s.tile([1, tn], F32, name="rcp")
                    nc.vector.reciprocal(rcp[:, :], rs[:, :])
                    rcp_r = b2s.tile([1, tn], F32R, name="rcp_r")
                    nc.vector.tensor_copy(rcp_r[:, :], rcp[:, :])
                    rcb = psa.tile([128, tn], F32, name="rcb")
                    nc.tensor.matmul(rcb[:, :], ones_row[0:1, :], rcp_r[:, :])
                    rcs = b2s.tile([128, tn], F32, name="rcs")
                    nc.scalar.activation(rcs[:, :], rcb[:, :], AF.Copy)
                    at = atp.tile([128, tn], BF16, name=f"attn{h}")
                    nc.vector.tensor_tensor(at[:, :], av[:, :], rcs[:, :], ALU.mult)
                    attn_by_chunk.setdefault((t0, t1, s), []).append(at)
                    if len(attn_by_chunk[(t0, t1, s)]) == HPC:
                        emit_proj(t0, t1, s, attn_by_chunk.pop((t0, t1, s)))

                def emit_proj(t0, t1, s, attn_t):
                    tn = t1 - t0
                    pofat = b2s.tile([128, KC * tn], BF16, name="pofat")
                    for o in range(KC):
                        pj = psp.tile([128, tn], F32, name="pj")
                        for hc in range(HPC):
                            nc.tensor.matmul(pj[:, :],
                                             pw[(s, hc)][:, o * 128:(o + 1) * 128],
                                             attn_t[hc][:, :],
                                             start=(hc == 0), stop=(hc == HPC - 1))
                        nc.scalar.activation(pofat[:, o * tn:(o + 1) * tn],
                                             pj[:, :], AF.Copy)
                    nc.sync.dma_start(pp3[:, :, t0:t1], pofat[:, :])

                units = [(t0, t1, s, h) for (t0, t1, s) in TCH for h in range(HPC)]
                pending = None
                for u in units:
                    pf = emit_scores(u)
                    if pending is not None:
                        emit_tail(*pending)
                    pending = (u, pf)
                emit_tail(*pending)
    nc.compile()
    return nc


# --------------------------------------------------------------------------
# Launch C: reduce proj partials + gate + residual + LN2 (token-sharded)
# --------------------------------------------------------------------------
def build_C():
    nc = _new_nc()
    pil = nc.dram_tensor("pil", [HID, NCORES * TOK], BF16, kind="ExternalInput").ap()
    xtc = nc.dram_tensor("xtc", [HID, TOK], F32, kind="ExternalInput").ap()
    vecs = {}
    for nm in ("g1_t", "g1_i", "pb_t", "pb_i", "sc2_t", "sc2_i", "sh2_t", "sh2_i"):
        vecs[nm] = nc.dram_tensor(nm, [128, KC], F32, kind="ExternalInput").ap()
    onesc_d = nc.dram_tensor("onesc", [128, 1], F32R, kind="ExternalInput").ap()
    onesr_d = nc.dram_tensor("onesr", [128, 128], F32R, kind="ExternalInput").ap()
    xmod2o = nc.dram_tensor("xmod2", [HID, TOK], BF16, kind="ExternalOutput").ap()
    x2o = nc.dram_tensor("x2", [HID, TOK], F32, kind="ExternalOutput").ap()

    CR = [(0, TOK_T, 't'), (TOK_T, TOK, 'i')]

    with tile.TileContext(nc) as tc:
        with tc.tile_pool(name="const", bufs=1) as cp, \
             tc.tile_pool(name="x2keep", bufs=1) as xk, \
             tc.tile_pool(name="work", bufs=2) as wk, \
             tc.tile_pool(name="ps", bufs=1, space="PSUM") as ps, \
             tc.tile_pool(name="ps1", bufs=1, space="PSUM") as ps1:
            ones_col = cp.tile([128, 1], F32R)
            nc.sync.dma_start(ones_col[:, :], onesc_d[:, :])
            ones_row = cp.tile([128, 128], F32R)
            nc.sync.dma_start(ones_row[:, :], onesr_d[:, :])
            vt = {}
            for nm, ap in vecs.items():
                t_ = cp.tile([128, KC], F32, name=nm)
                nc.sync.dma_start(t_[:, :], ap[:, :])
                vt[nm] = t_
            for s in "ti":
                nc.vector.tensor_scalar(vt[f"sc2_{s}"][:, :], vt[f"sc2_{s}"][:, :],
                                        1.0, None, ALU.add)

            x2t = [xk.tile([128, TOK], F32, name=f"x2_{k}") for k in range(KC)]
            ssum = ps1.tile([1, TOK], F32, name="ssum")
            ssq = ps1.tile([1, TOK], F32, name="ssq")
            for k in range(KC):
                acc = wk.tile([128, NCORES * TOK], BF16, name="acc")
                nc.sync.dma_start(acc[:, :], pil[k * 128:(k + 1) * 128, :])
                a0 = wk.tile([128, TOK], F32, name="a0")
                t0_ = wk.tile([128, TOK], BF16, name="t0_")
                t1_ = wk.tile([128, TOK], BF16, name="t1_")
                t2_ = wk.tile([128, TOK], BF16, name="t2_")
                t3_ = wk.tile([128, TOK], BF16, name="t3_")
                nc.vector.tensor_tensor(t0_[:, :], acc[:, 0:TOK], acc[:, TOK:2 * TOK], ALU.add)
                nc.vector.tensor_tensor(t1_[:, :], acc[:, 2 * TOK:3 * TOK], acc[:, 3 * TOK:4 * TOK], ALU.add)
                nc.vector.tensor_tensor(t2_[:, :], acc[:, 4 * TOK:5 * TOK], acc[:, 5 * TOK:6 * TOK], ALU.add)
                nc.vector.tensor_tensor(t3_[:, :], acc[:, 6 * TOK:7 * TOK], acc[:, 7 * TOK:8 * TOK], ALU.add)
                nc.vector.tensor_tensor(t0_[:, :], t0_[:, :], t1_[:, :], ALU.add)
                nc.vector.tensor_tensor(t2_[:, :], t2_[:, :], t3_[:, :], ALU.add)
                nc.vector.tensor_tensor(a0[:, :], t0_[:, :], t2_[:, :], ALU.add)
                xr = wk.tile([128, TOK], F32, name="xr")
                nc.sync.dma_start(xr[:, :], xtc[k * 128:(k + 1) * 128, :])
                # x2 = x + g1*(acc + pb), per stream column range
                for (c0, c1_, s) in CR:
                    nc.vector.tensor_scalar(a0[:, c0:c1_], a0[:, c0:c1_],
                                            vt[f"pb_{s}"][:, k:k + 1],
                                            vt[f"g1_{s}"][:, k:k + 1], ALU.add, ALU.mult)
                nc.vector.tensor_tensor(x2t[k][:, :], xr[:, :], a0[:, :], ALU.add)
                nc.sync.dma_start(x2o[k * 128:(k + 1) * 128, :], x2t[k][:, :])
                # LN2 stats accumulation (rounded copies feed the PE)
                x2r = wk.tile([128, TOK], F32R, name="x2r")
                nc.vector.tensor_copy(x2r[:, :], x2t[k][:, :])
                sq = wk.tile([128, TOK], F32R, name="sq")
                nc.scalar.activation(sq[:, :], x2t[k][:, :], AF.Square)
                nc.tensor.matmul(ssum[:, :], ones_col[:, :], x2r[:, :],
                                 start=(k == 0), stop=(k == KC - 1))
                nc.tensor.matmul(ssq[:, :], ones_col[:, :], sq[:, :],
                                 start=(k == 0), stop=(k == KC - 1))
            mu = cp.tile([1, TOK], F32)
            var = cp.tile([1, TOK], F32)
            mu2 = cp.tile([1, TOK], F32)
            nc.scalar.activation(mu[:, :], ssum[:, :], AF.Identity, scale=1.0 / HID)
            nc.vector.tensor_tensor(mu2[:, :], mu[:, :], mu[:, :], ALU.mult)
            nc.scalar.activation(var[:, :], ssq[:, :], AF.Identity, scale=1.0 / HID)
            nc.vector.tensor_tensor(var[:, :], var[:, :], mu2[:, :], ALU.subtract)
            rstd = cp.tile([1, TOK], F32)
            _emit_rsqrt(nc, cp, rstd[:, :], var[:, :], 1.0, EPS, "ln2")
            mur = cp.tile([1, TOK], F32)
            nc.vector.tensor_tensor(mur[:, :], mu[:, :], rstd[:, :], ALU.mult)
            rstd_r = cp.tile([1, TOK], F32R)
            mur_r = cp.tile([1, TOK], F32R)
            nc.vector.tensor_copy(rstd_r[:, :], rstd[:, :])
            nc.vector.tensor_copy(mur_r[:, :], mur[:, :])
            arep = ps.tile([128, TOK], F32, name="arep")
            brep = ps.tile([128, TOK], F32, name="brep")
            nc.tensor.matmul(arep[:, :], ones_row[0:1, :], rstd_r[:, :])
            nc.tensor.matmul(brep[:, :], ones_row[0:1, :], mur_r[:, :])
            areps = cp.tile([128, TOK], F32)
            breps = cp.tile([128, TOK], F32)
            nc.scalar.activation(areps[:, :], arep[:, :], AF.Copy)
            nc.scalar.activation(breps[:, :], brep[:, :], AF.Copy)
            for k in range(KC):
                xm = wk.tile([128, TOK], F32, name="xm")
                xmb = wk.tile([128, TOK], BF16, name="xmb")
                nc.vector.tensor_tensor(xm[:, :], x2t[k][:, :], areps[:, :], ALU.mult)
                nc.vector.tensor_tensor(xm[:, :], xm[:, :], breps[:, :], ALU.subtract)
                for (c0, c1_, s) in CR:
                    nc.vector.tensor_scalar(xmb[:, c0:c1_], xm[:, c0:c1_],
                                            vt[f"sc2_{s}"][:, k:k + 1],
                                            vt[f"sh2_{s}"][:, k:k + 1], ALU.mult, ALU.add)
                nc.sync.dma_start(xmod2o[k * 128:(k + 1) * 128, :], xmb[:, :])
    nc.compile()
    return nc


# --------------------------------------------------------------------------
# Launch D: MLP partial (mlp-hidden sharded 8x)
# --------------------------------------------------------------------------
def build_D():
    nc = _new_nc()
    xm2 = nc.dram_tensor("xm2", [HID, L_LIVE], BF16, kind="ExternalInput").ap()
    w0 = {}; b0 = {}; w2 = {}
    for s in "ti":
        w0[s] = nc.dram_tensor(f"w0_{s}", [HID, MC * 128], BF16, kind="ExternalInput").ap()
        b0[s] = nc.dram_tensor(f"b0_{s}", [128, MC], F32, kind="ExternalInput").ap()
        w2[s] = nc.dram_tensor(f"w2_{s}", [MC * 128, HID], BF16, kind="ExternalInput").ap()
    pp2 = nc.dram_tensor("pp2", [HID, L_LIVE], BF16, kind="ExternalOutput").ap()
    xm3 = xm2.rearrange("(k p) t -> p k t", p=128)
    pp3 = pp2.rearrange("(o p) t -> p o t", p=128)

    with tile.TileContext(nc) as tc:
        with tc.tile_pool(name="wts", bufs=1) as wp, \
             tc.tile_pool(name="wk", bufs=2) as wk, \
             tc.tile_pool(name="out", bufs=2) as op_, \
             tc.tile_pool(name="psa", bufs=3, space="PSUM") as psa, \
             tc.tile_pool(name="psb", bufs=3, space="PSUM") as psb:
            w0t = {}; w2t = {}; b0t = {}

            def load_weights(s):
                b_ = op_.tile([128, MC], F32, name=f"b0t{s}", bufs=1)
                nc.sync.dma_start(b_[:, :], b0[s][:, :])
                b0t[s] = b_
                lst = []
                for k in range(KC):
                    w_ = wp.tile([128, MC * 128], BF16, name=f"w0{s}_{k}")
                    nc.sync.dma_start(w_[:, :], w0[s][k * 128:(k + 1) * 128, :])
                    lst.append(w_)
                w0t[s] = lst
                lst2 = []
                for m in range(MC):
                    w_ = wp.tile([128, HID], BF16, name=f"w2{s}_{m}")
                    nc.sync.dma_start(w_[:, :], w2[s][m * 128:(m + 1) * 128, :])
                    lst2.append(w_)
                w2t[s] = lst2

            load_weights('i')
            for ci, (t0, t1, s) in enumerate(TCH[1:] + TCH[:1]):   # img first
                tn = t1 - t0
                xfat = wk.tile([128, KC * tn], BF16, name="xfat")
                nc.sync.dma_start(xfat[:, :], xm3[:, :, t0:t1])
                if ci == 1:
                    load_weights('t')   # txt weights stream during img compute
                hfat = wk.tile([128, MC * tn], BF16, name="hfat", bufs=1)
                for m in range(MC):
                    ph = psa.tile([128, tn], F32, name="ph")
                    for k in range(KC):
                        nc.tensor.matmul(ph[:, :],
                                         w0t[s][k][:, m * 128:(m + 1) * 128],
                                         xfat[:, k * tn:(k + 1) * tn],
                                         start=(k == 0), stop=(k == KC - 1))
                    nc.scalar.activation(hfat[:, m * tn:(m + 1) * tn], ph[:, :],
                                         AF.Gelu_apprx_tanh, bias=b0t[s][:, m:m + 1])
                pofat = op_.tile([128, KC * tn], BF16, name="pofat", bufs=1)
                for o in range(KC):
                    po = psb.tile([128, tn], F32, name="po")
                    for m in range(MC):
                        nc.tensor.matmul(po[:, :],
                                         w2t[s][m][:, o * 128:(o + 1) * 128],
                                         hfat[:, m * tn:(m + 1) * tn],
                                         start=(m == 0), stop=(m == MC - 1))
                    nc.scalar.activation(pofat[:, o * tn:(o + 1) * tn], po[:, :], AF.Copy)
                nc.sync.dma_start(pp3[:, :, t0:t1], pofat[:, :])
    nc.compile()
    return nc


# --------------------------------------------------------------------------
# Launch E: reduce MLP partials + gate + residual (token-sharded)
# --------------------------------------------------------------------------
def build_E():
    nc = _new_nc()
    pil2 = nc.dram_tensor("pil2", [HID, NCORES * TOK], BF16, kind="ExternalInput").ap()
    x2c = nc.dram_tensor("x2c", [HID, TOK], F32, kind="ExternalInput").ap()
    vecs = {}
    for nm in ("g2_t", "g2_i", "b2_t", "b2_i"):
        vecs[nm] = nc.dram_tensor(nm, [128, KC], F32, kind="ExternalInput").ap()
    outc = nc.dram_tensor("outc", [HID, TOK], F32, kind="ExternalOutput").ap()

    CR = [(0, TOK_T, 't'), (TOK_T, TOK, 'i')]
    with tile.TileContext(nc) as tc:
        with tc.tile_pool(name="const", bufs=1) as cp, \
             tc.tile_pool(name="work", bufs=2) as wk:
            vt = {}
            for nm, ap in vecs.items():
                t_ = cp.tile([128, KC], F32, name=nm)
                nc.sync.dma_start(t_[:, :], ap[:, :])
                vt[nm] = t_
            for k in range(KC):
                acc = wk.tile([128, NCORES * TOK], BF16, name="acc")
                nc.sync.dma_start(acc[:, :], pil2[k * 128:(k + 1) * 128, :])
                a0 = wk.tile([128, TOK], F32, name="a0")
                t0_ = wk.tile([128, TOK], BF16, name="t0_")
                t1_ = wk.tile([128, TOK], BF16, name="t1_")
                t2_ = wk.tile([128, TOK], BF16, name="t2_")
                t3_ = wk.tile([128, TOK], BF16, name="t3_")
                nc.vector.tensor_tensor(t0_[:, :], acc[:, 0:TOK], acc[:, TOK:2 * TOK], ALU.add)
                nc.vector.tensor_tensor(t1_[:, :], acc[:, 2 * TOK:3 * TOK], acc[:, 3 * TOK:4 * TOK], ALU.add)
                nc.vector.tensor_tensor(t2_[:, :], acc[:, 4 * TOK:5 * TOK], acc[:, 5 * TOK:6 * TOK], ALU.add)
                nc.vector.tensor_tensor(t3_[:, :], acc[:, 6 * TOK:7 * TOK], acc[:, 7 * TOK:8 * TOK], ALU.add)
                nc.vector.tensor_tensor(t0_[:, :], t0_[:, :], t1_[:, :], ALU.add)
                nc.vector.tensor_tensor(t2_[:, :], t2_[:, :], t3_[:, :], ALU.add)
                nc.vector.tensor_tensor(a0[:, :], t0_[:, :], t2_[:, :], ALU.add)
                xr = wk.tile([128, TOK], F32, name="xr")
                nc.sync.dma_start(xr[:, :], x2c[k * 128:(k + 1) * 128, :])
                for (c0, c1_, s) in CR:
                    nc.vector.tensor_scalar(a0[:, c0:c1_], a0[:, c0:c1_],
                                            vt[f"b2_{s}"][:, k:k + 1],
                                            vt[f"g2_{s}"][:, k:k + 1], ALU.add, ALU.mult)
                ot = wk.tile([128, TOK], F32, name="ot")
                nc.vector.tensor_tensor(ot[:, :], xr[:, :], a0[:, :], ALU.add)
                nc.sync.dma_start(outc[k * 128:(k + 1) * 128, :], ot[:, :])
    nc.compile()
    return nc


# --------------------------------------------------------------------------
# Host orchestration
# --------------------------------------------------------------------------
_BUILT = {}

# test-harness hooks: when PROFILE is set (by test.py), every launch is traced
# and its exec_time_ns is appended to EXEC_TIMES as (label, ns).
PROFILE = False
EXEC_TIMES = []


def _get(name, builder):
    if name not in _BUILT:
        _BUILT[name] = builder()
    return _BUILT[name]


def _run(nc, in_maps, label="?", **kw):
    res = bass_utils.run_bass_kernel_spmd(nc, in_maps, core_ids=list(range(NCORES)),
                                          trace=PROFILE, **kw)
    if PROFILE:
        EXEC_TIMES.append((label, res.exec_time_ns))
    return res


def _f(x):
    return np.ascontiguousarray(x, dtype=np.float32)


def _r(x):
    """Round to the fp32r (tf32-like) grid; returns float32 bits."""
    y = static_cast_fp32_to_fp32r(_f(x))
    return np.asarray(y).view(np.float32).reshape(np.shape(x))


def _bf(x):
    return np.ascontiguousarray(np.asarray(x, np.float32).astype(ml_dtypes.bfloat16))


def _pp_vec(v):
    """[2048] vector -> [128, 16] per-partition scalar layout."""
    return _f(np.asarray(v).reshape(KC, 128).T)


def _own_idx(c):
    """canonical token indices owned by core c (32 txt + 256 img)."""
    return np.concatenate([np.arange(c * TOK_T, (c + 1) * TOK_T),
                           L_TXT + np.arange(c * TOK_I, (c + 1) * TOK_I)])


def kernel(**inputs):
    img = np.asarray(inputs["img"], np.float32)
    txt = np.asarray(inputs["txt"], np.float32)
    vec = np.asarray(inputs["vec"], np.float32)
    pe = np.asarray(inputs["pe"], np.float32)
    k_cache = np.asarray(inputs["k_cache"], np.float32)
    v_cache = np.asarray(inputs["v_cache"], np.float32)
    live = np.asarray(inputs["live_indices"]).astype(np.int64)

    # feature-major activations, txt tokens first
    x_t = _f(np.concatenate([txt[0].T, img[0].T], axis=1))       # [2048, 2304]
    x_t_b = _bf(x_t)
    vec_t = _f(vec[0][:, None])
    ones_c = np.ones((128, 1), np.float32)
    ones_r = np.ones((128, 128), np.float32)

    mw = {s: np.asarray(inputs[f"{p}_mod_w"], np.float32)
          for s, p in (('i', 'img'), ('t', 'txt'))}
    mb = {s: np.asarray(inputs[f"{p}_mod_b"], np.float32)
          for s, p in (('i', 'img'), ('t', 'txt'))}

    # ---------------- Launch A (sh1/sc1 GEMV + LN1 stats) ----------------
    ncA = _get("A", build_A)
    in_maps = []
    for c in range(NCORES):
        cols = []
        bcols = []
        rs = np.arange(c * 256, (c + 1) * 256)
        for s in "it":
            for blk in (0, 1):           # sh1, sc1
                rows = blk * HID + rs
                cols.append(mw[s][rows].T)
                bcols.append(mb[s][rows][None, :])
        statx = np.empty((512, HID), np.float32)
        statx[0:256] = img[0, c * 256:(c + 1) * 256]
        statx[256:512] = txt[0]
        in_maps.append({
            "vec_t": vec_t,
            "modw": _bf(np.concatenate(cols, axis=1)),
            "modb": _f(np.concatenate(bcols, axis=1)),
            "statx": _bf(statx),
        })
    resA = _run(ncA, in_maps, label="A")

    mod = {}
    # modo layout per core: [img_sh1, img_sc1, txt_sh1, txt_sc1] x 256
    for bi, (s, nm) in enumerate((('i', 'sh1'), ('i', 'sc1'),
                                  ('t', 'sh1'), ('t', 'sc1'))):
        mod[f"{nm}_{s}"] = np.concatenate(
            [resA.results[c]["modo"][0, bi * 256:(bi + 1) * 256] for c in range(NCORES)])
    rstd_img = np.concatenate([resA.results[c]["stato"][0, 0:256] for c in range(NCORES)])
    murd_img = np.concatenate([resA.results[c]["stato"][1, 0:256] for c in range(NCORES)])
    rstd_txt = resA.results[0]["stato"][0, 256:512]
    murd_txt = resA.results[0]["stato"][1, 256:512]
    arow = np.concatenate([rstd_txt, rstd_img])      # [2304]
    brow = np.concatenate([murd_txt, murd_img])
    arep = _f(np.broadcast_to(arow[None, :], (128, L_LIVE)))
    brep = _f(np.broadcast_to(brow[None, :], (128, L_LIVE)))

    # pe -> PEa/PEb feature-major [128 d, 2304]
    pe0 = pe[0, 0]                                   # [2304, 64, 2, 2]
    dd = np.arange(D)
    pea = _f(pe0[:, dd // 2, dd % 2, 0].T)
    peb = _f(pe0[:, dd // 2, dd % 2, 1].T)

    # S matrices with qknorm scale folded (host: indexing only)
    smats = {}
    for s, pref in (('t', 'txt'), ('i', 'img')):
        qn = np.asarray(inputs[f"{pref}_qnorm"], np.float32)
        kn = np.asarray(inputs[f"{pref}_knorm"], np.float32)
        for m, nv, off in (("sevq", qn, 0), ("sodq", qn, 1),
                           ("sevk", kn, 0), ("sodk", kn, 1)):
            S = np.zeros((128, 128), np.float32)
            src = 2 * (dd // 2) + off
            S[src, dd] = nv[src]
            smats[f"{m}_{s}"] = S

    dead = np.setdiff1d(np.arange(L_FULL), live)
    ident = _f(np.eye(128))

    # ---------------- Launch B ----------------
    ncB = _get("B", build_B)
    in_maps = []
    for c in range(NCORES):
        h0 = HPC * c
        qkv = {}
        for s, pref in (('t', 'txt'), ('i', 'img')):
            W = np.asarray(inputs[f"{pref}_qkv_w"], np.float32)
            bq = np.asarray(inputs[f"{pref}_qkv_b"], np.float32)
            rows = []
            brows = []
            for part in range(3):            # q, k, v
                for h in (h0, h0 + 1):
                    rows.append(W[part * HID + h * D: part * HID + (h + 1) * D])
                    brows.append(bq[part * HID + h * D: part * HID + (h + 1) * D])
            qkv[f"qkvw_{s}"] = _bf(np.concatenate(rows, axis=0).T)         # [2048, 768]
            qkv[f"qkvb_{s}"] = _f(np.stack(brows, axis=1))                 # [128, 6]
            P = np.asarray(inputs[f"{pref}_proj_w"], np.float32)
            qkv[f"projw_{s}"] = _bf(P[:, h0 * D:(h0 + HPC) * D].T)         # [256, 2048]
        kd = np.concatenate([k_cache[0, h, dead, :].T for h in (h0, h0 + 1)], axis=0)
        vd = np.concatenate([v_cache[0, h, dead, :] for h in (h0, h0 + 1)], axis=1)
        # mod2 GEMV slice: [img g1/sh2/sc2/g2, txt g1/sh2/sc2/g2] x 256
        cols = []
        bcols = []
        rs = np.arange(c * 256, (c + 1) * 256)
        for s in "it":
            for blk in (2, 3, 4, 5):     # g1, sh2, sc2, g2
                rows2 = blk * HID + rs
                cols.append(mw[s][rows2].T)
                bcols.append(mb[s][rows2][None, :])
        m = {
            "xt": x_t_b, "arep": arep, "brep": brep, "pea": pea, "peb": peb,
            "ident": ident, "kdead": _r(kd), "vdead": _bf(vd),
            "onesc": ones_c, "onescb": _bf(ones_c), "onesr": ones_r, "vec_t": vec_t,
            "modw2": _bf(np.concatenate(cols, axis=1)),
            "modb2": _f(np.concatenate(bcols, axis=1)),
        }
        m.update(qkv)
        for s in "ti":
            m[f"sc1_{s}"] = _pp_vec(mod[f"sc1_{s}"])
            m[f"sh1_{s}"] = _bf(_pp_vec(mod[f"sh1_{s}"]))
        m.update({k: _r(v) for k, v in smats.items()})
        in_maps.append(m)
    resB = _run(ncB, in_maps, label="B")
    partials = [resB.results[c]["pp"] for c in range(NCORES)]    # bf16 [2048, 2304]
    # mod2 reassembly
    for bi, (s, nm) in enumerate((('i', 'g1'), ('i', 'sh2'), ('i', 'sc2'), ('i', 'g2'),
                                  ('t', 'g1'), ('t', 'sh2'), ('t', 'sc2'), ('t', 'g2'))):
        mod[f"{nm}_{s}"] = np.concatenate(
            [resB.results[c]["modo2"][0, bi * 256:(bi + 1) * 256] for c in range(NCORES)])
    kernel._dbg = {"mod": mod, "arow": arow, "brow": brow,
                   "partials": [np.asarray(p, np.float32) for p in partials]}

    # ---------------- Launch C ----------------
    ncC = _get("C", build_C)
    in_maps = []
    for c in range(NCORES):
        idx = _own_idx(c)
        pil = np.stack([p[:, idx] for p in partials], axis=1)    # [2048, 8, 288] bf16
        m = {"pil": np.ascontiguousarray(pil.reshape(HID, NCORES * TOK)),
             "xtc": _f(x_t[:, idx]),
             "onesc": ones_c, "onesr": ones_r}
        for s, pref in (('t', 'txt'), ('i', 'img')):
            m[f"g1_{s}"] = _pp_vec(mod[f"g1_{s}"])
            m[f"pb_{s}"] = _pp_vec(inputs[f"{pref}_proj_b"])
            m[f"sc2_{s}"] = _pp_vec(mod[f"sc2_{s}"])
            m[f"sh2_{s}"] = _pp_vec(mod[f"sh2_{s}"])
        in_maps.append(m)
    resC = _run(ncC, in_maps, label="C")

    xmod2 = np.empty((HID, L_LIVE), ml_dtypes.bfloat16)
    x2 = np.empty((HID, L_LIVE), np.float32)
    for c in range(NCORES):
        idx = _own_idx(c)
        xmod2[:, idx] = resC.results[c]["xmod2"]
        x2[:, idx] = resC.results[c]["x2"]

    # ---------------- Launch D ----------------
    ncD = _get("D", build_D)
    xm2_b = np.ascontiguousarray(xmod2)
    in_maps = []
    for c in range(NCORES):
        rows = slice(c * (MLP // NCORES), (c + 1) * (MLP // NCORES))
        m = {"xm2": xm2_b}
        for s, pref in (('t', 'txt'), ('i', 'img')):
            W0 = np.asarray(inputs[f"{pref}_mlp0_w"], np.float32)
            B0 = np.asarray(inputs[f"{pref}_mlp0_b"], np.float32)
            W2 = np.asarray(inputs[f"{pref}_mlp2_w"], np.float32)
            m[f"w0_{s}"] = _bf(W0[rows].T)                       # [2048, 1024]
            m[f"b0_{s}"] = _f(B0[rows].reshape(MC, 128).T)       # [128, 8]
            m[f"w2_{s}"] = _bf(W2[:, rows].T)                    # [1024, 2048]
        in_maps.append(m)
    resD = _run(ncD, in_maps, label="D")
    partials2 = [resD.results[c]["pp2"] for c in range(NCORES)]
    kernel._dbg.update({"xmod2": np.asarray(xmod2, np.float32), "x2": x2,
                        "partials2": [np.asarray(p, np.float32) for p in partials2]})

    # ---------------- Launch E ----------------
    ncE = _get("E", build_E)
    in_maps = []
    for c in range(NCORES):
        idx = _own_idx(c)
        pil2 = np.stack([p[:, idx] for p in partials2], axis=1)
        m = {"pil2": np.ascontiguousarray(pil2.reshape(HID, NCORES * TOK)),
             "x2c": _f(x2[:, idx])}
        for s, pref in (('t', 'txt'), ('i', 'img')):
            m[f"g2_{s}"] = _pp_vec(mod[f"g2_{s}"])
            m[f"b2_{s}"] = _pp_vec(inputs[f"{pref}_mlp2_b"])
        in_maps.append(m)
    resE = _run(ncE, in_maps, label="E")

    out = np.empty((HID, L_LIVE), np.float32)
    for c in range(NCORES):
        out[:, _own_idx(c)] = resE.results[c]["outc"]
    out_tok = out.T                                              # [2304, 2048]
    txt_out = np.ascontiguousarray(out_tok[:L_TXT])[None, :, :]
    img_out = np.ascontiguousarray(out_tok[L_TXT:])[None, :, :]
    return (img_out.astype(np.float32), txt_out.astype(np.float32))


# revision 30
# speedup vs baseline: 1.0088x; 1.0088x over previous
"""Self-contained Trainium2 Bass kernel for the Flux-style DoubleStreamBlock.

Strategy (8 NeuronCores, SPMD via run_bass_kernel_spmd):
  Launch A: modulation GEMV for the (sh1, sc1) chunks, sharded over output
            rows, + LayerNorm-1 token statistics (token-sharded).
  Launch B: modulation GEMV for the remaining chunks, then head-parallel
            (2 heads/core) QKV + QKNorm + RoPE + attention over the full KV
            cache + row-parallel proj partial sums.
            - LayerNorm-1 + modulation are folded into the QKV matmul:
              qkv[o,t] = rstd[t]*(W'^T x)[o,t] - (mu*rstd)[t]*s[o] + d[o]+b[o]
              with W' = W*(1+sc1)[k], s = col-sums of W', d = W^T sh1.
              So x feeds the PE straight from DMA with no vector-engine prep.
            - The cache scatter is eliminated by reordering keys: softmax is
              permutation-invariant over key positions, so each core gets the
              768 "dead" cache rows (host-gathered) plus the 2304 fresh k/v.
  Launch C: token-sharded reduction of proj partials + gate/residual + LN2.
  Launch D: MLP with the 8192-dim hidden sharded 8x (Megatron-style).
  Launch E: token-sharded reduction of MLP partials + gate/residual = output.

The host only slices / transposes / concatenates / broadcasts numpy arrays
between launches (plus fp32->fp32r / fp32->bf16 format casts); every
arithmetic op of the reference runs on-device.

dtype scheme: moving (rhs) matmul operands are float32r (TF32-like, full PE
rate at moving-dim >= 256, pre-rounded or written via an f32r-typed AP so
the walrus fp32r verifier is satisfied). Stationary weight operands are
bf16 (fast weight load, half the DMA). Attention k/v/q stay f32r for
precision. Cross-core partial sums travel as bf16.
"""

import numpy as np
import ml_dtypes

import concourse.bacc as bacc
import concourse.mybir as mybir
import concourse.tile as tile
from concourse import bass_utils
from neuron_dtypes import static_cast_fp32_to_fp32r

dt = mybir.dt
AF = mybir.ActivationFunctionType
ALU = mybir.AluOpType
F32 = dt.float32
F32R = dt.float32r
BF16 = dt.bfloat16

# Problem shapes (hardcoded per contract).
L_TXT, L_IMG = 256, 2048
L_LIVE, L_FULL = 2304, 3072
HID, H, D, MLP = 2048, 16, 128, 8192
EPS = 1e-6
NCORES = 8
HPC = H // NCORES            # heads per core = 2
L_DEAD = L_FULL - L_LIVE     # 768
KC = HID // 128              # 16 hidden-dim chunks
MC = MLP // NCORES // 128    # 8 mlp-hidden chunks per core
SC = L_FULL // 128           # 24 key-position chunks
INV_SQRT_D = float(1.0 / np.sqrt(D))

# token chunks (stream-aligned: txt tokens first, then img)
TCH = [(0, 256, 't'), (256, 768, 'i'), (768, 1280, 'i'),
       (1280, 1792, 'i'), (1792, 2304, 'i')]

# ownership split for token-sharded launches C/E: core c owns
# txt[c*32:(c+1)*32] + img[c*256:(c+1)*256]  -> uniform 32/256 col split.
TOK_T, TOK_I = L_TXT // NCORES, L_IMG // NCORES   # 32, 256
TOK = TOK_T + TOK_I                               # 288


def _new_nc():
    return bacc.Bacc("TRN2", target_bir_lowering=False, debug=False,
                     enable_asserts=True, num_devices=NCORES)


def _f32(ap):
    return ap.bitcast(F32)


def _emit_rsqrt(nc, pool, out, in_ap, scale, bias, name):
    """out = 1/sqrt(in*scale + bias), via ACT Sqrt + DVE reciprocal + one
    Newton polish (ACT Sqrt's spline budget is loose)."""
    p, f = out.partition_size(), out.free_size()
    v = pool.tile([p, f], F32, name=f"{name}_v")
    s = pool.tile([p, f], F32, name=f"{name}_s")
    t = pool.tile([p, f], F32, name=f"{name}_t")
    b = pool.tile([p, 1], F32, name=f"{name}_b")
    nc.vector.memset(b[:, :], bias)
    # v = in*scale + bias ; s = sqrt(v)
    nc.scalar.activation(v[:, :], in_ap, AF.Identity, bias=b[:, 0:1], scale=scale)
    nc.scalar.activation(s[:, :], in_ap, AF.Sqrt, bias=b[:, 0:1], scale=scale)
    nc.vector.reciprocal(out, s[:, :])                      # y ~ rsqrt(v)
    # Newton: y' = y*(1.5 - 0.5*v*y^2)
    nc.vector.tensor_tensor(t[:, :], out, out, ALU.mult)    # y^2
    nc.vector.tensor_tensor(t[:, :], t[:, :], v[:, :], ALU.mult)
    nc.vector.tensor_scalar(t[:, :], t[:, :], -0.5, 1.5, ALU.mult, ALU.add)
    nc.vector.tensor_tensor(out, out, t[:, :], ALU.mult)


def _emit_mod_gemv(nc, pools, silu, modw_ap, modb_ap, ncols, out_ap):
    """out[1, ncols] = silu^T @ modw + modb   (modw bf16 [HID, ncols])."""
    sp, wp, pp = pools
    btile = sp.tile([1, ncols], F32, name="modbt")
    nc.sync.dma_start(btile[:, :], modb_ap[:, :])
    nch = ncols // 512
    psums = [pp.tile([1, 512], F32, name=f"gmm{o}") for o in range(nch)]
    for k in range(KC):
        wt = wp.tile([128, ncols], BF16, name="gwt")
        nc.sync.dma_start(wt[:, :], modw_ap[k * 128:(k + 1) * 128, :])
        for o in range(nch):
            nc.tensor.matmul(psums[o][:, :], silu[:, k:k + 1],
                             wt[:, o * 512:(o + 1) * 512],
                             start=(k == 0), stop=(k == KC - 1))
    mout = sp.tile([1, ncols], F32, name="gmout")
    for o in range(nch):
        nc.vector.tensor_tensor(mout[:, o * 512:(o + 1) * 512], psums[o][:, :],
                                btile[:, o * 512:(o + 1) * 512], ALU.add)
    nc.sync.dma_start(out_ap[:, :], mout[:, :])


# --------------------------------------------------------------------------
# Launch A: modulation GEMV for sh1/sc1 (row-sharded) + LN1 stats
# --------------------------------------------------------------------------
def build_A():
    nc = _new_nc()
    vec_t = nc.dram_tensor("vec_t", [HID, 1], F32, kind="ExternalInput").ap()
    modw = nc.dram_tensor("modw", [HID, 1024], BF16, kind="ExternalInput").ap()
    modb = nc.dram_tensor("modb", [1, 1024], F32, kind="ExternalInput").ap()
    statx = nc.dram_tensor("statx", [512, HID], BF16, kind="ExternalInput").ap()
    modo = nc.dram_tensor("modo", [1, 1024], F32, kind="ExternalOutput").ap()
    stato = nc.dram_tensor("stato", [2, 512], F32, kind="ExternalOutput").ap()

    with tile.TileContext(nc) as tc:
        with tc.tile_pool(name="small", bufs=1) as sp, \
             tc.tile_pool(name="wts", bufs=3) as wp, \
             tc.tile_pool(name="stat", bufs=2) as st, \
             tc.tile_pool(name="ps", bufs=1, space="PSUM") as pp:
            vraw = sp.tile([128, KC], F32)
            nc.sync.dma_start(vraw[:, :], vec_t.rearrange("(c p) x -> p (c x)", p=128))
            silu = sp.tile([128, KC], BF16)
            nc.scalar.activation(silu[:, :], vraw[:, :], AF.Silu)
            _emit_mod_gemv(nc, (sp, wp, pp), silu, modw, modb, 1024, modo)

            # ---- LN1 stats on 512 tokens (token-major, batched rsqrt) ----
            mu4 = sp.tile([128, 4], F32)
            var4 = sp.tile([128, 4], F32)
            for i in range(4):
                xt = st.tile([128, HID], BF16, name="xt")
                nc.sync.dma_start(xt[:, :], statx[i * 128:(i + 1) * 128, :])
                sq = st.tile([128, HID], F32, name="sq")
                nc.scalar.activation(sq[:, :], xt[:, :], AF.Square)
                s1 = st.tile([128, 1], F32, name="s1")
                s2 = st.tile([128, 1], F32, name="s2")
                nc.vector.tensor_reduce(s1[:, :], xt[:, :], mybir.AxisListType.X, ALU.add)
                nc.vector.tensor_reduce(s2[:, :], sq[:, :], mybir.AxisListType.X, ALU.add)
                nc.vector.tensor_scalar(mu4[:, i:i + 1], s1[:, :], 1.0 / HID, None, ALU.mult)
                nc.vector.tensor_scalar(var4[:, i:i + 1], s2[:, :], 1.0 / HID, None, ALU.mult)
            mu2 = sp.tile([128, 4], F32)
            nc.vector.tensor_tensor(mu2[:, :], mu4[:, :], mu4[:, :], ALU.mult)
            nc.vector.tensor_tensor(var4[:, :], var4[:, :], mu2[:, :], ALU.subtract)
            rstd4 = sp.tile([128, 4], F32)
            _emit_rsqrt(nc, sp, rstd4[:, :], var4[:, :], 1.0, EPS, "rs")
            murstd4 = sp.tile([128, 4], F32)
            nc.vector.tensor_tensor(murstd4[:, :], mu4[:, :], rstd4[:, :], ALU.mult)
            for i in range(4):
                nc.sync.dma_start(stato[0:1, i * 128:(i + 1) * 128], rstd4[:, i:i + 1])
                nc.sync.dma_start(stato[1:2, i * 128:(i + 1) * 128], murstd4[:, i:i + 1])
    nc.compile()
    return nc


# --------------------------------------------------------------------------
# Launch B: mod2 GEMV + QKV(+LN1 fold) + QKNorm + RoPE + attention + proj
# --------------------------------------------------------------------------
def build_B():
    nc = _new_nc()
    inp = {}

    def din(name, shape, d=F32):
        inp[name] = nc.dram_tensor(name, shape, d, kind="ExternalInput").ap()

    din("vec_t", [HID, 1])
    din("modw2", [HID, 2048], BF16)
    din("modb2", [1, 2048])
    din("xt", [HID, L_LIVE], BF16)
    din("qkvw_t", [HID, 6 * 128], BF16); din("qkvw_i", [HID, 6 * 128], BF16)
    din("qkvb_t", [128, 6]); din("qkvb_i", [128, 6])
    din("projw_t", [HPC * 128, HID], BF16); din("projw_i", [HPC * 128, HID], BF16)
    din("sc1_t", [128, KC]); din("sh1_t", [128, KC], BF16)
    din("sc1_i", [128, KC]); din("sh1_i", [128, KC], BF16)
    din("arep", [128, L_LIVE]); din("brep", [128, L_LIVE])
    din("pea", [128, L_LIVE]); din("peb", [128, L_LIVE])
    for s in "ti":
        for m in ("sevq", "sodq", "sevk", "sodk"):
            din(f"{m}_{s}", [128, 128], F32R)
    din("ident", [128, 128])
    din("onesc", [128, 1], F32R)
    din("onescb", [128, 1], BF16)
    din("onesr", [128, 128], F32R)
    din("kdead", [HPC * 128, L_DEAD], F32R)
    din("vdead", [L_DEAD, HPC * 128], BF16)
    pp_out = nc.dram_tensor("pp", [HID, L_LIVE], BF16, kind="ExternalOutput").ap()
    modo2 = nc.dram_tensor("modo2", [1, 2048], F32, kind="ExternalOutput").ap()

    with tile.TileContext(nc) as tc:
        with tc.tile_pool(name="const", bufs=1) as cp, \
             tc.tile_pool(name="state", bufs=1) as stp:
            ones_col = cp.tile([128, 1], F32R)
            nc.sync.dma_start(ones_col[:, :], inp["onesc"][:, :])
            ones_colb = cp.tile([128, 1], BF16)
            nc.sync.dma_start(ones_colb[:, :], inp["onescb"][:, :])
            ones_row = cp.tile([128, 128], F32R)
            nc.sync.dma_start(ones_row[:, :], inp["onesr"][:, :])
            ident = cp.tile([128, 128], F32)
            nc.sync.dma_start(ident[:, :], inp["ident"][:, :])
            smat = {}
            for s in "ti":
                for m in ("sevq", "sodq", "sevk", "sodk"):
                    t_ = cp.tile([128, 128], F32R, name=f"{m}{s}")
                    nc.sync.dma_start(t_[:, :], inp[f"{m}_{s}"][:, :])
                    smat[(m, s)] = t_
            c1 = {}; sh1 = {}; qkvb = {}
            for s in "ti":
                c = cp.tile([128, KC], F32, name=f"c1{s}")
                nc.sync.dma_start(c[:, :], inp[f"sc1_{s}"][:, :])
                nc.vector.tensor_scalar(c[:, :], c[:, :], 1.0, None, ALU.add)
                c1[s] = c
                hh = cp.tile([128, KC], BF16, name=f"sh1{s}")
                nc.sync.dma_start(hh[:, :], inp[f"sh1_{s}"][:, :])
                sh1[s] = hh
                bb = cp.tile([128, 6], F32, name=f"qb{s}")
                nc.sync.dma_start(bb[:, :], inp[f"qkvb_{s}"][:, :])
                qkvb[s] = bb

            # persistent attention state (per head)
            kall = [stp.tile([128, L_FULL], F32R, name=f"kall{h}") for h in range(HPC)]
            vfull = [stp.tile([128, L_FULL], BF16, name=f"vfull{h}") for h in range(HPC)]
            qrot = [stp.tile([128, L_LIVE], F32R, name=f"qrot{h}") for h in range(HPC)]
            for h in range(HPC):
                nc.sync.dma_start(kall[h][:, 0:L_DEAD],
                                  inp["kdead"][h * 128:(h + 1) * 128, :])
                for c6 in range(L_DEAD // 128):
                    nc.sync.dma_start(
                        vfull[h][:, c6 * 128:(c6 + 1) * 128],
                        inp["vdead"][c6 * 128:(c6 + 1) * 128, h * 128:(h + 1) * 128])

            # ---------------- B1: QKV (LN folded) + qknorm + rope ----------
            for s, chunks in (('t', TCH[:1]), ('i', TCH[1:])):
                with tc.tile_pool(name=f"w{s}", bufs=1) as wpool, \
                     tc.tile_pool(name=f"b1{s}", bufs=2) as b1p, \
                     tc.tile_pool(name=f"b1s{s}", bufs=1) as b1small:
                    wq = []
                    for k in range(KC):
                        w_ = wpool.tile([128, 6 * 128], BF16, name=f"wq{k}")
                        nc.sync.dma_start(w_[:, :], inp[f"qkvw_{s}"][k * 128:(k + 1) * 128, :])
                        wq.append(w_)
                    # d[o] = W^T sh1 (before W' overwrite); then W'=W*(1+sc1);
                    # s[o] = col-sums of W'; svec = -s; bias2_j = d + qkv_b
                    svec = b1small.tile([128, 6], F32, name="svec")
                    bias2 = b1small.tile([128, 6], F32, name="bias2")
                    with tc.tile_pool(name=f"psd{s}", bufs=2, space="PSUM") as psd:
                        for j in range(6):
                            dp = psd.tile([128, 1], F32, name="dp")
                            for k in range(KC):
                                nc.tensor.matmul(dp[:, :],
                                                 wq[k][:, j * 128:(j + 1) * 128],
                                                 sh1[s][:, k:k + 1],
                                                 start=(k == 0), stop=(k == KC - 1))
                            nc.vector.tensor_tensor(bias2[:, j:j + 1], dp[:, :],
                                                    qkvb[s][:, j:j + 1], ALU.add)
                        # W' = W * (1+sc1)[k]  (in place)
                        for k in range(KC):
                            nc.vector.tensor_scalar(wq[k][:, :], wq[k][:, :],
                                                    c1[s][:, k:k + 1], None, ALU.mult)
                        for j in range(6):
                            sp_ = psd.tile([128, 1], F32, name="sp_")
                            for k in range(KC):
                                nc.tensor.matmul(sp_[:, :],
                                                 wq[k][:, j * 128:(j + 1) * 128],
                                                 ones_colb[:, :],
                                                 start=(k == 0), stop=(k == KC - 1))
                            nc.vector.tensor_scalar(svec[:, j:j + 1], sp_[:, :],
                                                    -1.0, None, ALU.mult)
                    with tc.tile_pool(name=f"psq{s}", bufs=2, space="PSUM") as psq, \
                         tc.tile_pool(name=f"pse{s}", bufs=1, space="PSUM") as pse, \
                         tc.tile_pool(name=f"psx{s}", bufs=1, space="PSUM") as psx:
                      xt3 = inp["xt"].rearrange("(k p) t -> p k t", p=128)

                      def emit_qkv(t0, t1):
                        tn = t1 - t0
                        xfat = b1p.tile([128, KC * tn], BF16, name="xfat")
                        nc.sync.dma_start(xfat[:, :], xt3[:, :, t0:t1])
                        arep_t = b1small.tile([128, tn], F32, name="arep_t", bufs=2)
                        brep_t = b1small.tile([128, tn], F32, name="brep_t", bufs=2)
                        pea_t = b1small.tile([128, tn], F32, name="pea_t", bufs=2)
                        peb_t = b1small.tile([128, tn], F32, name="peb_t", bufs=2)
                        nc.sync.dma_start(arep_t[:, :], inp["arep"][:, t0:t1])
                        nc.sync.dma_start(brep_t[:, :], inp["brep"][:, t0:t1])
                        nc.sync.dma_start(pea_t[:, :], inp["pea"][:, t0:t1])
                        nc.sync.dma_start(peb_t[:, :], inp["peb"][:, t0:t1])
                        # QKV: G = W'^T x ; out = a[t]*G + (-s[o]*b[t] + bias2[o])
                        raw = []
                        for j in range(6):
                            pq = psq.tile([128, tn], F32, name="pq")
                            for k in range(KC):
                                nc.tensor.matmul(
                                    pq[:, :], wq[k][:, j * 128:(j + 1) * 128],
                                    xfat[:, k * tn:(k + 1) * tn],
                                    start=(k == 0), stop=(k == KC - 1))
                            r = b1p.tile([128, tn], F32R if j < 4 else F32,
                                         name=f"raw{j}", bufs=2)
                            r1 = b1small.tile([128, tn], F32, name="r1")
                            nc.vector.tensor_scalar(r1[:, :], brep_t[:, :],
                                                    svec[:, j:j + 1],
                                                    bias2[:, j:j + 1],
                                                    ALU.mult, ALU.add)
                            e1 = b1small.tile([128, tn], F32, name="e1")
                            nc.vector.tensor_tensor(e1[:, :], pq[:, :],
                                                    arep_t[:, :], ALU.mult)
                            nc.vector.tensor_tensor(r[:, :], e1[:, :], r1[:, :],
                                                    ALU.add)
                            raw.append(r)
                        return (t0, t1, raw, pea_t, peb_t)

                      def emit_tail(t0, t1, raw, pea_t, peb_t):
                        tn = t1 - t0
                        # v: transpose into vfull rows
                        for h in range(HPC):
                            for j1 in range(tn // 128):
                                tp = psx.tile([128, 128], F32, name="tp")
                                nc.tensor.transpose(tp[:, :],
                                                    raw[4 + h][:, j1 * 128:(j1 + 1) * 128],
                                                    ident[:, :])
                                col0 = L_DEAD + t0 + j1 * 128
                                nc.scalar.activation(vfull[h][:, col0:col0 + 128],
                                                     tp[:, :], AF.Copy)
                        # q/k: rmsnorm stats (batched over the 4 q/k tiles)
                        QK = [(h, w) for h in range(HPC) for w in ('q', 'k')]
                        rs4 = b1small.tile([128, tn], F32, name="rs4", bufs=2)
                        for r, (h, w) in enumerate(QK):
                            src_ = raw[h] if w == 'q' else raw[2 + h]
                            sqt = b1small.tile([128, tn], F32R, name="sqt", bufs=2)
                            nc.scalar.activation(sqt[:, :], _f32(src_[:, :]), AF.Square)
                            rsum = psx.tile([1, tn], F32, name="rsum", bufs=2)
                            nc.tensor.matmul(rsum[:, :], ones_col[:, :], sqt[:, :])
                            nc.scalar.activation(rs4[32 * r:32 * r + 1, :], rsum[:, :], AF.Copy)
                        rr4 = b1small.tile([128, tn], F32, name="rr4", bufs=2)
                        _emit_rsqrt(nc, b1small, rr4[:, :], rs4[:, :],
                                    1.0 / D, EPS, "rr")
                        rr4_r = b1small.tile([128, tn], F32R, name="rr4_r", bufs=2)
                        nc.vector.tensor_copy(rr4_r[:, :], rr4[:, :])
                        # rope: dst = (PEa*(Sev q) + PEb*(Sod q)) * rrms
                        for r, (h, w) in enumerate(QK):
                            mev, mod_ = ("sevq", "sodq") if w == 'q' else ("sevk", "sodk")
                            src_ = raw[h] if w == 'q' else raw[2 + h]
                            rrb = psx.tile([128, tn], F32, name="rrb", bufs=1)
                            nc.tensor.matmul(rrb[:, :], ones_row[32 * r:32 * r + 1, :],
                                             rr4_r[32 * r:32 * r + 1, :],
                                             tile_position=(32 * r, 0))
                            ev = pse.tile([128, tn], F32, name="ev")
                            od = pse.tile([128, tn], F32, name="od")
                            nc.tensor.matmul(ev[:, :], smat[(mev, s)][:, :], src_[:, :])
                            nc.tensor.matmul(od[:, :], smat[(mod_, s)][:, :], src_[:, :])
                            t1_ = b1small.tile([128, tn], F32, name="t1_", bufs=2)
                            t2_ = b1small.tile([128, tn], F32, name="t2_", bufs=2)
                            nc.vector.tensor_tensor(t1_[:, :], ev[:, :], pea_t[:, :], ALU.mult)
                            nc.vector.tensor_tensor(t2_[:, :], od[:, :], peb_t[:, :], ALU.mult)
                            nc.vector.tensor_tensor(t1_[:, :], t1_[:, :], t2_[:, :], ALU.add)
                            if w == 'q':
                                dst = qrot[h][:, t0:t1]
                            else:
                                dst = kall[h][:, L_DEAD + t0:L_DEAD + t1]
                            nc.vector.tensor_tensor(dst, t1_[:, :], rrb[:, :], ALU.mult)

                      pend = None
                      for (t0, t1, _s) in chunks:
                        st_ = emit_qkv(t0, t1)
                        if pend is not None:
                            emit_tail(*pend)
                        pend = st_
                      emit_tail(*pend)

            # ---- mod2 GEMV (PE covers B1->B2 transition) ----
            with tc.tile_pool(name="gsm", bufs=1) as gsp, \
                 tc.tile_pool(name="gw", bufs=3) as gwp, \
                 tc.tile_pool(name="gps", bufs=1, space="PSUM") as gpp:
                vraw = gsp.tile([128, KC], F32)
                nc.sync.dma_start(vraw[:, :],
                                  inp["vec_t"].rearrange("(c p) x -> p (c x)", p=128))
                silu = gsp.tile([128, KC], BF16)
                nc.scalar.activation(silu[:, :], vraw[:, :], AF.Silu)
                _emit_mod_gemv(nc, (gsp, gwp, gpp), silu, inp["modw2"],
                               inp["modb2"], 2048, modo2)

            # ---------------- B2+B3: attention + proj partial ----------------
            with tc.tile_pool(name="projw", bufs=1) as pwp, \
                 tc.tile_pool(name="pfat", bufs=2) as pfp, \
                 tc.tile_pool(name="attnp", bufs=2) as atp, \
                 tc.tile_pool(name="b2small", bufs=2) as b2s, \
                 tc.tile_pool(name="pss", bufs=2, space="PSUM") as pss, \
                 tc.tile_pool(name="psa", bufs=1, space="PSUM") as psa, \
                 tc.tile_pool(name="psp", bufs=1, space="PSUM") as psp:
                pw = {}
                for s in "ti":
                    for hc in range(HPC):
                        w_ = pwp.tile([128, HID], BF16, name=f"pw{s}{hc}")
                        nc.sync.dma_start(w_[:, :],
                                          inp[f"projw_{s}"][hc * 128:(hc + 1) * 128, :])
                        pw[(s, hc)] = w_
                pp3 = pp_out.rearrange("(o p) t -> p o t", p=128)
                attn_by_chunk = {}

                def emit_scores(u):
                    (t0, t1, s, h) = u
                    tn = t1 - t0
                    pfat = pfp.tile([128, SC * tn], BF16, name="pfat")
                    for sc2 in range(SC // 2):
                        ps_ = pss.tile([128, 2 * tn], F32, name="ps_")
                        for half in range(2):
                            sc = 2 * sc2 + half
                            nc.tensor.matmul(ps_[:, half * tn:(half + 1) * tn],
                                             kall[h][:, sc * 128:(sc + 1) * 128],
                                             qrot[h][:, t0:t1])
                        nc.scalar.activation(pfat[:, 2 * sc2 * tn:(2 * sc2 + 2) * tn],
                                             ps_[:, :], AF.Exp, scale=INV_SQRT_D)
                    return pfat

                def emit_tail(u, pfat):
                    (t0, t1, s, h) = u
                    tn = t1 - t0
                    rs = psa.tile([1, tn], F32, name="rs")
                    for sc in range(SC):
                        nc.tensor.matmul(rs[:, :], ones_colb[:, :],
                                         pfat[:, sc * tn:(sc + 1) * tn],
                                         start=(sc == 0), stop=(sc == SC - 1))
                    av = psa.tile([128, tn], F32, name="av")
                    for sc in range(SC):
                        nc.tensor.matmul(av[:, :],
                                         vfull[h][:, sc * 128:(sc + 1) * 128],
                                         pfat[:, sc * tn:(sc + 1) * tn],
                                         start=(sc == 0), stop=(sc == SC - 1))
                    rcp = b2s.tile([1, tn], F32, name="rcp")
                    nc.vector.reciprocal(rcp[:, :], rs[:, :])
                    rcp_r = b2s.tile([1, tn], F32R, name="rcp_r")
                    nc.vector.tensor_copy(rcp_r[:, :], rcp[:, :])
                    rcb = psa.tile([128, tn], F32, name="rcb")
                    nc.tensor.matmul(rcb[:, :], ones_row[0:1, :], rcp_r[:, :])
                    rcs = b2s.tile([128, tn], F32, name="rcs")
                    nc.scalar.activation(rcs[:, :], rcb[:, :], AF.Copy)
                    at = atp.tile([128, tn], BF16, name=f"attn{h}")
                    nc.vector.tensor_tensor(at[:, :], av[:, :], rcs[:, :], ALU.mult)
                    attn_by_chunk.setdefault((t0, t1, s), []).append(at)
                    if len(attn_by_chunk[(t0, t1, s)]) == HPC:
                        emit_proj(t0, t1, s, attn_by_chunk.pop((t0, t1, s)))

                def emit_proj(t0, t1, s, attn_t):
                    tn = t1 - t0
                    pofat = b2s.tile([128, KC * tn], BF16, name="pofat")
                    for o in range(KC):
                        pj = psp.tile([128, tn], F32, name="pj")
                        for hc in range(HPC):
                            nc.tensor.matmul(pj[:, :],
                                             pw[(s, hc)][:, o * 128:(o + 1) * 128],
                                             attn_t[hc][:, :],
                                             start=(hc == 0), stop=(hc == HPC - 1))
                        nc.scalar.activation(pofat[:, o * tn:(o + 1) * tn],
                                             pj[:, :], AF.Copy)
                    nc.sync.dma_start(pp3[:, :, t0:t1], pofat[:, :])

                units = [(t0, t1, s, h) for (t0, t1, s) in TCH[1:] + TCH[:1]
                         for h in range(HPC)]
                pending = None
                for u in units:
                    pf = emit_scores(u)
                    if pending is not None:
                        emit_tail(*pending)
                    pending = (u, pf)
                emit_tail(*pending)
    nc.compile()
    return nc


# --------------------------------------------------------------------------
# Launch C: reduce proj partials + gate + residual + LN2 (token-sharded)
# --------------------------------------------------------------------------
def build_C():
    nc = _new_nc()
    pil = nc.dram_tensor("pil", [HID, NCORES * TOK], BF16, kind="ExternalInput").ap()
    xtc = nc.dram_tensor("xtc", [HID, TOK], F32, kind="ExternalInput").ap()
    vecs = {}
    for nm in ("g1_t", "g1_i", "pb_t", "pb_i"):
        vecs[nm] = nc.dram_tensor(nm, [128, KC], F32, kind="ExternalInput").ap()
    onesc_d = nc.dram_tensor("onesc", [128, 1], F32R, kind="ExternalInput").ap()
    x2o = nc.dram_tensor("x2", [HID, TOK], F32, kind="ExternalOutput").ap()
    x2bo = nc.dram_tensor("x2b", [HID, TOK], BF16, kind="ExternalOutput").ap()
    stato = nc.dram_tensor("stato", [2, TOK], F32, kind="ExternalOutput").ap()

    CR = [(0, TOK_T, 't'), (TOK_T, TOK, 'i')]

    with tile.TileContext(nc) as tc:
        with tc.tile_pool(name="const", bufs=1) as cp, \
             tc.tile_pool(name="work", bufs=2) as wk, \
             tc.tile_pool(name="ps1", bufs=1, space="PSUM") as ps1:
            ones_col = cp.tile([128, 1], F32R)
            nc.sync.dma_start(ones_col[:, :], onesc_d[:, :])
            vt = {}
            for nm, ap in vecs.items():
                t_ = cp.tile([128, KC], F32, name=nm)
                nc.sync.dma_start(t_[:, :], ap[:, :])
                vt[nm] = t_

            ssum = ps1.tile([1, TOK], F32, name="ssum")
            ssq = ps1.tile([1, TOK], F32, name="ssq")
            for k in range(KC):
                acc = wk.tile([128, NCORES * TOK], BF16, name="acc")
                nc.sync.dma_start(acc[:, :], pil[k * 128:(k + 1) * 128, :])
                a0 = wk.tile([128, TOK], F32, name="a0")
                t0_ = wk.tile([128, TOK], BF16, name="t0_")
                t1_ = wk.tile([128, TOK], BF16, name="t1_")
                t2_ = wk.tile([128, TOK], BF16, name="t2_")
                t3_ = wk.tile([128, TOK], BF16, name="t3_")
                nc.vector.tensor_tensor(t0_[:, :], acc[:, 0:TOK], acc[:, TOK:2 * TOK], ALU.add)
                nc.vector.tensor_tensor(t1_[:, :], acc[:, 2 * TOK:3 * TOK], acc[:, 3 * TOK:4 * TOK], ALU.add)
                nc.vector.tensor_tensor(t2_[:, :], acc[:, 4 * TOK:5 * TOK], acc[:, 5 * TOK:6 * TOK], ALU.add)
                nc.vector.tensor_tensor(t3_[:, :], acc[:, 6 * TOK:7 * TOK], acc[:, 7 * TOK:8 * TOK], ALU.add)
                nc.vector.tensor_tensor(t0_[:, :], t0_[:, :], t1_[:, :], ALU.add)
                nc.vector.tensor_tensor(t2_[:, :], t2_[:, :], t3_[:, :], ALU.add)
                nc.vector.tensor_tensor(a0[:, :], t0_[:, :], t2_[:, :], ALU.add)
                xr = wk.tile([128, TOK], F32, name="xr")
                nc.sync.dma_start(xr[:, :], xtc[k * 128:(k + 1) * 128, :])
                # x2 = x + g1*(acc + pb), per stream column range
                for (c0, c1_, s) in CR:
                    nc.vector.tensor_scalar(a0[:, c0:c1_], a0[:, c0:c1_],
                                            vt[f"pb_{s}"][:, k:k + 1],
                                            vt[f"g1_{s}"][:, k:k + 1], ALU.add, ALU.mult)
                x2t = wk.tile([128, TOK], F32, name="x2t")
                nc.vector.tensor_tensor(x2t[:, :], xr[:, :], a0[:, :], ALU.add)
                nc.sync.dma_start(x2o[k * 128:(k + 1) * 128, :], x2t[:, :])
                # LN2 stats accumulation (rounded copies feed the PE)
                x2r = wk.tile([128, TOK], F32R, name="x2r")
                nc.vector.tensor_copy(x2r[:, :], x2t[:, :])
                x2b = wk.tile([128, TOK], BF16, name="x2b")
                nc.vector.tensor_copy(x2b[:, :], x2t[:, :])
                nc.sync.dma_start(x2bo[k * 128:(k + 1) * 128, :], x2b[:, :])
                sq = wk.tile([128, TOK], F32R, name="sq")
                nc.scalar.activation(sq[:, :], x2t[:, :], AF.Square)
                nc.tensor.matmul(ssum[:, :], ones_col[:, :], x2r[:, :],
                                 start=(k == 0), stop=(k == KC - 1))
                nc.tensor.matmul(ssq[:, :], ones_col[:, :], sq[:, :],
                                 start=(k == 0), stop=(k == KC - 1))
            mu = cp.tile([1, TOK], F32)
            var = cp.tile([1, TOK], F32)
            mu2 = cp.tile([1, TOK], F32)
            nc.scalar.activation(mu[:, :], ssum[:, :], AF.Identity, scale=1.0 / HID)
            nc.vector.tensor_tensor(mu2[:, :], mu[:, :], mu[:, :], ALU.mult)
            nc.scalar.activation(var[:, :], ssq[:, :], AF.Identity, scale=1.0 / HID)
            nc.vector.tensor_tensor(var[:, :], var[:, :], mu2[:, :], ALU.subtract)
            rstd = cp.tile([1, TOK], F32)
            _emit_rsqrt(nc, cp, rstd[:, :], var[:, :], 1.0, EPS, "ln2")
            mur = cp.tile([1, TOK], F32)
            nc.vector.tensor_tensor(mur[:, :], mu[:, :], rstd[:, :], ALU.mult)
            nc.sync.dma_start(stato[0:1, :], rstd[:, :])
            nc.sync.dma_start(stato[1:2, :], mur[:, :])
    nc.compile()
    return nc


# --------------------------------------------------------------------------
# Launch D: MLP partial (mlp-hidden sharded 8x)
# --------------------------------------------------------------------------
def build_D():
    nc = _new_nc()
    x2b = nc.dram_tensor("x2b", [HID, L_LIVE], BF16, kind="ExternalInput").ap()
    arep2 = nc.dram_tensor("arep2", [128, L_LIVE], F32, kind="ExternalInput").ap()
    brep2 = nc.dram_tensor("brep2", [128, L_LIVE], F32, kind="ExternalInput").ap()
    w0 = {}; b0 = {}; w2 = {}; sc2 = {}; sh2 = {}
    for s in "ti":
        w0[s] = nc.dram_tensor(f"w0_{s}", [HID, MC * 128], BF16, kind="ExternalInput").ap()
        b0[s] = nc.dram_tensor(f"b0_{s}", [128, MC], F32, kind="ExternalInput").ap()
        w2[s] = nc.dram_tensor(f"w2_{s}", [MC * 128, HID], BF16, kind="ExternalInput").ap()
        sc2[s] = nc.dram_tensor(f"sc2_{s}", [128, KC], F32, kind="ExternalInput").ap()
        sh2[s] = nc.dram_tensor(f"sh2_{s}", [128, KC], BF16, kind="ExternalInput").ap()
    onesc_d = nc.dram_tensor("onescb", [128, 1], BF16, kind="ExternalInput").ap()
    pp2 = nc.dram_tensor("pp2", [HID, L_LIVE], BF16, kind="ExternalOutput").ap()
    xm3 = x2b.rearrange("(k p) t -> p k t", p=128)
    pp3 = pp2.rearrange("(o p) t -> p o t", p=128)

    with tile.TileContext(nc) as tc:
        with tc.tile_pool(name="wts", bufs=1) as wp, \
             tc.tile_pool(name="cst", bufs=1) as cst, \
             tc.tile_pool(name="wk", bufs=2) as wk, \
             tc.tile_pool(name="out", bufs=2) as op_, \
             tc.tile_pool(name="psa", bufs=3, space="PSUM") as psa, \
             tc.tile_pool(name="psd", bufs=1, space="PSUM") as psd, \
             tc.tile_pool(name="psb", bufs=2, space="PSUM") as psb:
            ones_colb = cst.tile([128, 1], BF16)
            nc.sync.dma_start(ones_colb[:, :], onesc_d[:, :])
            w0t = {}; w2t = {}; b0t = {}; svec = {}; dvec = {}

            def load_weights(s):
                b_ = op_.tile([128, MC], F32, name=f"b0t{s}", bufs=1)
                nc.sync.dma_start(b_[:, :], b0[s][:, :])
                b0t[s] = b_
                c_ = cst.tile([128, KC], F32, name=f"c2{s}")
                nc.sync.dma_start(c_[:, :], sc2[s][:, :])
                nc.vector.tensor_scalar(c_[:, :], c_[:, :], 1.0, None, ALU.add)
                h_ = cst.tile([128, KC], BF16, name=f"sh2{s}")
                nc.sync.dma_start(h_[:, :], sh2[s][:, :])
                lst = []
                for k in range(KC):
                    w_ = wp.tile([128, MC * 128], BF16, name=f"w0{s}_{k}")
                    nc.sync.dma_start(w_[:, :], w0[s][k * 128:(k + 1) * 128, :])
                    lst.append(w_)
                w0t[s] = lst
                lst2 = []
                for m in range(MC):
                    w_ = wp.tile([128, HID], BF16, name=f"w2{s}_{m}")
                    nc.sync.dma_start(w_[:, :], w2[s][m * 128:(m + 1) * 128, :])
                    lst2.append(w_)
                w2t[s] = lst2
                # d0[m] = W0^T sh2, then W0' = W0*(1+sc2)[k], s0[m] = colsums
                sv = cst.tile([128, MC], F32, name=f"sv{s}")
                dv = cst.tile([128, MC], F32, name=f"dv{s}")
                for m in range(MC):
                    dp = psd.tile([128, 1], F32, name="dp")
                    for k in range(KC):
                        nc.tensor.matmul(dp[:, :], lst[k][:, m * 128:(m + 1) * 128],
                                         h_[:, k:k + 1],
                                         start=(k == 0), stop=(k == KC - 1))
                    nc.vector.tensor_tensor(dv[:, m:m + 1], dp[:, :],
                                            b0t[s][:, m:m + 1], ALU.add)
                for k in range(KC):
                    nc.vector.tensor_scalar(lst[k][:, :], lst[k][:, :],
                                            c_[:, k:k + 1], None, ALU.mult)
                for m in range(MC):
                    sp_ = psd.tile([128, 1], F32, name="sp_")
                    for k in range(KC):
                        nc.tensor.matmul(sp_[:, :], lst[k][:, m * 128:(m + 1) * 128],
                                         ones_colb[:, :],
                                         start=(k == 0), stop=(k == KC - 1))
                    nc.vector.tensor_scalar(sv[:, m:m + 1], sp_[:, :],
                                            -1.0, None, ALU.mult)
                svec[s] = sv; dvec[s] = dv

            load_weights('i')
            for ci, (t0, t1, s) in enumerate(TCH[1:] + TCH[:1]):   # img first
                tn = t1 - t0
                xfat = wk.tile([128, KC * tn], BF16, name="xfat")
                nc.sync.dma_start(xfat[:, :], xm3[:, :, t0:t1])
                at_ = wk.tile([128, tn], F32, name="at_", bufs=2)
                bt_ = wk.tile([128, tn], F32, name="bt_", bufs=2)
                nc.sync.dma_start(at_[:, :], arep2[:, t0:t1])
                nc.sync.dma_start(bt_[:, :], brep2[:, t0:t1])
                if ci == 1:
                    load_weights('t')   # txt weights stream during img compute
                hfat = wk.tile([128, MC * tn], BF16, name="hfat", bufs=1)
                for m in range(MC):
                    ph = psa.tile([128, tn], F32, name="ph")
                    for k in range(KC):
                        nc.tensor.matmul(ph[:, :],
                                         w0t[s][k][:, m * 128:(m + 1) * 128],
                                         xfat[:, k * tn:(k + 1) * tn],
                                         start=(k == 0), stop=(k == KC - 1))
                    # h = a[t]*G + (-s0[m]*b[t] + d0[m]+b0[m]) ; then gelu
                    r1 = wk.tile([128, tn], F32, name="r1", bufs=2)
                    nc.vector.tensor_scalar(r1[:, :], bt_[:, :],
                                            svec[s][:, m:m + 1],
                                            dvec[s][:, m:m + 1], ALU.mult, ALU.add)
                    e1 = wk.tile([128, tn], F32, name="e1", bufs=2)
                    nc.vector.tensor_tensor(e1[:, :], ph[:, :], at_[:, :], ALU.mult)
                    nc.vector.tensor_tensor(e1[:, :], e1[:, :], r1[:, :], ALU.add)
                    nc.scalar.activation(hfat[:, m * tn:(m + 1) * tn], e1[:, :],
                                         AF.Gelu_apprx_tanh)
                pofat = op_.tile([128, KC * tn], BF16, name="pofat", bufs=1)
                for o in range(KC):
                    po = psb.tile([128, tn], F32, name="po")
                    for m in range(MC):
                        nc.tensor.matmul(po[:, :],
                                         w2t[s][m][:, o * 128:(o + 1) * 128],
                                         hfat[:, m * tn:(m + 1) * tn],
                                         start=(m == 0), stop=(m == MC - 1))
                    nc.scalar.activation(pofat[:, o * tn:(o + 1) * tn], po[:, :], AF.Copy)
                nc.sync.dma_start(pp3[:, :, t0:t1], pofat[:, :])
    nc.compile()
    return nc


# --------------------------------------------------------------------------
# Launch E: reduce MLP partials + gate + residual (token-sharded)
# --------------------------------------------------------------------------
def build_E():
    nc = _new_nc()
    pil2 = nc.dram_tensor("pil2", [HID, NCORES * TOK], BF16, kind="ExternalInput").ap()
    x2c = nc.dram_tensor("x2c", [HID, TOK], F32, kind="ExternalInput").ap()
    vecs = {}
    for nm in ("g2_t", "g2_i", "b2_t", "b2_i"):
        vecs[nm] = nc.dram_tensor(nm, [128, KC], F32, kind="ExternalInput").ap()
    outc = nc.dram_tensor("outc", [HID, TOK], F32, kind="ExternalOutput").ap()

    CR = [(0, TOK_T, 't'), (TOK_T, TOK, 'i')]
    with tile.TileContext(nc) as tc:
        with tc.tile_pool(name="const", bufs=1) as cp, \
             tc.tile_pool(name="work", bufs=2) as wk:
            vt = {}
            for nm, ap in vecs.items():
                t_ = cp.tile([128, KC], F32, name=nm)
                nc.sync.dma_start(t_[:, :], ap[:, :])
                vt[nm] = t_
            for k in range(KC):
                acc = wk.tile([128, NCORES * TOK], BF16, name="acc")
                nc.sync.dma_start(acc[:, :], pil2[k * 128:(k + 1) * 128, :])
                a0 = wk.tile([128, TOK], F32, name="a0")
                t0_ = wk.tile([128, TOK], BF16, name="t0_")
                t1_ = wk.tile([128, TOK], BF16, name="t1_")
                t2_ = wk.tile([128, TOK], BF16, name="t2_")
                t3_ = wk.tile([128, TOK], BF16, name="t3_")
                nc.vector.tensor_tensor(t0_[:, :], acc[:, 0:TOK], acc[:, TOK:2 * TOK], ALU.add)
                nc.vector.tensor_tensor(t1_[:, :], acc[:, 2 * TOK:3 * TOK], acc[:, 3 * TOK:4 * TOK], ALU.add)
                nc.vector.tensor_tensor(t2_[:, :], acc[:, 4 * TOK:5 * TOK], acc[:, 5 * TOK:6 * TOK], ALU.add)
                nc.vector.tensor_tensor(t3_[:, :], acc[:, 6 * TOK:7 * TOK], acc[:, 7 * TOK:8 * TOK], ALU.add)
                nc.vector.tensor_tensor(t0_[:, :], t0_[:, :], t1_[:, :], ALU.add)
                nc.vector.tensor_tensor(t2_[:, :], t2_[:, :], t3_[:, :], ALU.add)
                nc.vector.tensor_tensor(a0[:, :], t0_[:, :], t2_[:, :], ALU.add)
                xr = wk.tile([128, TOK], F32, name="xr")
                nc.sync.dma_start(xr[:, :], x2c[k * 128:(k + 1) * 128, :])
                for (c0, c1_, s) in CR:
                    nc.vector.tensor_scalar(a0[:, c0:c1_], a0[:, c0:c1_],
                                            vt[f"b2_{s}"][:, k:k + 1],
                                            vt[f"g2_{s}"][:, k:k + 1], ALU.add, ALU.mult)
                ot = wk.tile([128, TOK], F32, name="ot")
                nc.vector.tensor_tensor(ot[:, :], xr[:, :], a0[:, :], ALU.add)
                nc.sync.dma_start(outc[k * 128:(k + 1) * 128, :], ot[:, :])
    nc.compile()
    return nc


# --------------------------------------------------------------------------
# Host orchestration
# --------------------------------------------------------------------------
_BUILT = {}

# test-harness hooks: when PROFILE is set (by test.py), every launch is traced
# and its exec_time_ns is appended to EXEC_TIMES as (label, ns).
PROFILE = False
EXEC_TIMES = []


def _get(name, builder):
    if name not in _BUILT:
        _BUILT[name] = builder()
    return _BUILT[name]


def _run(nc, in_maps, label="?", **kw):
    res = bass_utils.run_bass_kernel_spmd(nc, in_maps, core_ids=list(range(NCORES)),
                                          trace=PROFILE, **kw)
    if PROFILE:
        EXEC_TIMES.append((label, res.exec_time_ns))
    return res


def _f(x):
    return np.ascontiguousarray(x, dtype=np.float32)


def _r(x):
    """Round to the fp32r (tf32-like) grid; returns float32 bits."""
    y = static_cast_fp32_to_fp32r(_f(x))
    return np.asarray(y).view(np.float32).reshape(np.shape(x))


def _bf(x):
    return np.ascontiguousarray(np.asarray(x, np.float32).astype(ml_dtypes.bfloat16))


def _pp_vec(v):
    """[2048] vector -> [128, 16] per-partition scalar layout."""
    return _f(np.asarray(v).reshape(KC, 128).T)


def _own_idx(c):
    """canonical token indices owned by core c (32 txt + 256 img)."""
    return np.concatenate([np.arange(c * TOK_T, (c + 1) * TOK_T),
                           L_TXT + np.arange(c * TOK_I, (c + 1) * TOK_I)])


def kernel(**inputs):
    img = np.asarray(inputs["img"], np.float32)
    txt = np.asarray(inputs["txt"], np.float32)
    vec = np.asarray(inputs["vec"], np.float32)
    pe = np.asarray(inputs["pe"], np.float32)
    k_cache = np.asarray(inputs["k_cache"], np.float32)
    v_cache = np.asarray(inputs["v_cache"], np.float32)
    live = np.asarray(inputs["live_indices"]).astype(np.int64)

    # feature-major activations, txt tokens first
    x_t = _f(np.concatenate([txt[0].T, img[0].T], axis=1))       # [2048, 2304]
    x_t_b = _bf(x_t)
    vec_t = _f(vec[0][:, None])
    ones_c = np.ones((128, 1), np.float32)
    ones_r = np.ones((128, 128), np.float32)

    mw = {s: np.asarray(inputs[f"{p}_mod_w"], np.float32)
          for s, p in (('i', 'img'), ('t', 'txt'))}
    mb = {s: np.asarray(inputs[f"{p}_mod_b"], np.float32)
          for s, p in (('i', 'img'), ('t', 'txt'))}

    # ---------------- Launch A (sh1/sc1 GEMV + LN1 stats) ----------------
    ncA = _get("A", build_A)
    in_maps = []
    for c in range(NCORES):
        cols = []
        bcols = []
        rs = np.arange(c * 256, (c + 1) * 256)
        for s in "it":
            for blk in (0, 1):           # sh1, sc1
                rows = blk * HID + rs
                cols.append(mw[s][rows].T)
                bcols.append(mb[s][rows][None, :])
        statx = np.empty((512, HID), np.float32)
        statx[0:256] = img[0, c * 256:(c + 1) * 256]
        statx[256:512] = txt[0]
        in_maps.append({
            "vec_t": vec_t,
            "modw": _bf(np.concatenate(cols, axis=1)),
            "modb": _f(np.concatenate(bcols, axis=1)),
            "statx": _bf(statx),
        })
    resA = _run(ncA, in_maps, label="A")

    mod = {}
    # modo layout per core: [img_sh1, img_sc1, txt_sh1, txt_sc1] x 256
    for bi, (s, nm) in enumerate((('i', 'sh1'), ('i', 'sc1'),
                                  ('t', 'sh1'), ('t', 'sc1'))):
        mod[f"{nm}_{s}"] = np.concatenate(
            [resA.results[c]["modo"][0, bi * 256:(bi + 1) * 256] for c in range(NCORES)])
    rstd_img = np.concatenate([resA.results[c]["stato"][0, 0:256] for c in range(NCORES)])
    murd_img = np.concatenate([resA.results[c]["stato"][1, 0:256] for c in range(NCORES)])
    rstd_txt = resA.results[0]["stato"][0, 256:512]
    murd_txt = resA.results[0]["stato"][1, 256:512]
    arow = np.concatenate([rstd_txt, rstd_img])      # [2304]
    brow = np.concatenate([murd_txt, murd_img])
    arep = _f(np.broadcast_to(arow[None, :], (128, L_LIVE)))
    brep = _f(np.broadcast_to(brow[None, :], (128, L_LIVE)))

    # pe -> PEa/PEb feature-major [128 d, 2304]
    pe0 = pe[0, 0]                                   # [2304, 64, 2, 2]
    dd = np.arange(D)
    pea = _f(pe0[:, dd // 2, dd % 2, 0].T)
    peb = _f(pe0[:, dd // 2, dd % 2, 1].T)

    # S matrices with qknorm scale folded (host: indexing only)
    smats = {}
    for s, pref in (('t', 'txt'), ('i', 'img')):
        qn = np.asarray(inputs[f"{pref}_qnorm"], np.float32)
        kn = np.asarray(inputs[f"{pref}_knorm"], np.float32)
        for m, nv, off in (("sevq", qn, 0), ("sodq", qn, 1),
                           ("sevk", kn, 0), ("sodk", kn, 1)):
            S = np.zeros((128, 128), np.float32)
            src = 2 * (dd // 2) + off
            S[src, dd] = nv[src]
            smats[f"{m}_{s}"] = S

    dead = np.setdiff1d(np.arange(L_FULL), live)
    ident = _f(np.eye(128))

    # ---------------- Launch B ----------------
    ncB = _get("B", build_B)
    in_maps = []
    for c in range(NCORES):
        h0 = HPC * c
        qkv = {}
        for s, pref in (('t', 'txt'), ('i', 'img')):
            W = np.asarray(inputs[f"{pref}_qkv_w"], np.float32)
            bq = np.asarray(inputs[f"{pref}_qkv_b"], np.float32)
            rows = []
            brows = []
            for part in range(3):            # q, k, v
                for h in (h0, h0 + 1):
                    rows.append(W[part * HID + h * D: part * HID + (h + 1) * D])
                    brows.append(bq[part * HID + h * D: part * HID + (h + 1) * D])
            qkv[f"qkvw_{s}"] = _bf(np.concatenate(rows, axis=0).T)         # [2048, 768]
            qkv[f"qkvb_{s}"] = _f(np.stack(brows, axis=1))                 # [128, 6]
            P = np.asarray(inputs[f"{pref}_proj_w"], np.float32)
            qkv[f"projw_{s}"] = _bf(P[:, h0 * D:(h0 + HPC) * D].T)         # [256, 2048]
        kd = np.concatenate([k_cache[0, h, dead, :].T for h in (h0, h0 + 1)], axis=0)
        vd = np.concatenate([v_cache[0, h, dead, :] for h in (h0, h0 + 1)], axis=1)
        # mod2 GEMV slice: [img g1/sh2/sc2/g2, txt g1/sh2/sc2/g2] x 256
        cols = []
        bcols = []
        rs = np.arange(c * 256, (c + 1) * 256)
        for s in "it":
            for blk in (2, 3, 4, 5):     # g1, sh2, sc2, g2
                rows2 = blk * HID + rs
                cols.append(mw[s][rows2].T)
                bcols.append(mb[s][rows2][None, :])
        m = {
            "xt": x_t_b, "arep": arep, "brep": brep, "pea": pea, "peb": peb,
            "ident": ident, "kdead": _r(kd), "vdead": _bf(vd),
            "onesc": ones_c, "onescb": _bf(ones_c), "onesr": ones_r, "vec_t": vec_t,
            "modw2": _bf(np.concatenate(cols, axis=1)),
            "modb2": _f(np.concatenate(bcols, axis=1)),
        }
        m.update(qkv)
        for s in "ti":
            m[f"sc1_{s}"] = _pp_vec(mod[f"sc1_{s}"])
            m[f"sh1_{s}"] = _bf(_pp_vec(mod[f"sh1_{s}"]))
        m.update({k: _r(v) for k, v in smats.items()})
        in_maps.append(m)
    resB = _run(ncB, in_maps, label="B")
    partials = [resB.results[c]["pp"] for c in range(NCORES)]    # bf16 [2048, 2304]
    # mod2 reassembly
    for bi, (s, nm) in enumerate((('i', 'g1'), ('i', 'sh2'), ('i', 'sc2'), ('i', 'g2'),
                                  ('t', 'g1'), ('t', 'sh2'), ('t', 'sc2'), ('t', 'g2'))):
        mod[f"{nm}_{s}"] = np.concatenate(
            [resB.results[c]["modo2"][0, bi * 256:(bi + 1) * 256] for c in range(NCORES)])
    kernel._dbg = {"mod": mod, "arow": arow, "brow": brow,
                   "partials": [np.asarray(p, np.float32) for p in partials]}

    # ---------------- Launch C ----------------
    ncC = _get("C", build_C)
    in_maps = []
    for c in range(NCORES):
        idx = _own_idx(c)
        pil = np.stack([p[:, idx] for p in partials], axis=1)    # [2048, 8, 288] bf16
        m = {"pil": np.ascontiguousarray(pil.reshape(HID, NCORES * TOK)),
             "xtc": _f(x_t[:, idx]),
             "onesc": ones_c}
        for s, pref in (('t', 'txt'), ('i', 'img')):
            m[f"g1_{s}"] = _pp_vec(mod[f"g1_{s}"])
            m[f"pb_{s}"] = _pp_vec(inputs[f"{pref}_proj_b"])
        in_maps.append(m)
    resC = _run(ncC, in_maps, label="C")

    x2b = np.empty((HID, L_LIVE), ml_dtypes.bfloat16)
    x2 = np.empty((HID, L_LIVE), np.float32)
    rstd2 = np.empty(L_LIVE, np.float32)
    murd2 = np.empty(L_LIVE, np.float32)
    for c in range(NCORES):
        idx = _own_idx(c)
        x2b[:, idx] = resC.results[c]["x2b"]
        x2[:, idx] = resC.results[c]["x2"]
        rstd2[idx] = resC.results[c]["stato"][0]
        murd2[idx] = resC.results[c]["stato"][1]
    arep2 = _f(np.broadcast_to(rstd2[None, :], (128, L_LIVE)))
    brep2 = _f(np.broadcast_to(murd2[None, :], (128, L_LIVE)))

    # ---------------- Launch D ----------------
    ncD = _get("D", build_D)
    x2b_c = np.ascontiguousarray(x2b)
    in_maps = []
    for c in range(NCORES):
        rows = slice(c * (MLP // NCORES), (c + 1) * (MLP // NCORES))
        m = {"x2b": x2b_c, "arep2": arep2, "brep2": brep2,
             "onescb": _bf(ones_c)}
        for s, pref in (('t', 'txt'), ('i', 'img')):
            W0 = np.asarray(inputs[f"{pref}_mlp0_w"], np.float32)
            B0 = np.asarray(inputs[f"{pref}_mlp0_b"], np.float32)
            W2 = np.asarray(inputs[f"{pref}_mlp2_w"], np.float32)
            m[f"w0_{s}"] = _bf(W0[rows].T)                       # [2048, 1024]
            m[f"b0_{s}"] = _f(B0[rows].reshape(MC, 128).T)       # [128, 8]
            m[f"w2_{s}"] = _bf(W2[:, rows].T)                    # [1024, 2048]
            m[f"sc2_{s}"] = _pp_vec(mod[f"sc2_{s}"])
            m[f"sh2_{s}"] = _bf(_pp_vec(mod[f"sh2_{s}"]))
        in_maps.append(m)
    resD = _run(ncD, in_maps, label="D")
    partials2 = [resD.results[c]["pp2"] for c in range(NCORES)]
    kernel._dbg.update({"x2": x2,
                        "partials2": [np.asarray(p, np.float32) for p in partials2]})

    # ---------------- Launch E ----------------
    ncE = _get("E", build_E)
    in_maps = []
    for c in range(NCORES):
        idx = _own_idx(c)
        pil2 = np.stack([p[:, idx] for p in partials2], axis=1)
        m = {"pil2": np.ascontiguousarray(pil2.reshape(HID, NCORES * TOK)),
             "x2c": _f(x2[:, idx])}
        for s, pref in (('t', 'txt'), ('i', 'img')):
            m[f"g2_{s}"] = _pp_vec(mod[f"g2_{s}"])
            m[f"b2_{s}"] = _pp_vec(inputs[f"{pref}_mlp2_b"])
        in_maps.append(m)
    resE = _run(ncE, in_maps, label="E")

    out = np.empty((HID, L_LIVE), np.float32)
    for c in range(NCORES):
        out[:, _own_idx(c)] = resE.results[c]["outc"]
    out_tok = out.T                                              # [2304, 2048]
    txt_out = np.ascontiguousarray(out_tok[:L_TXT])[None, :, :]
    img_out = np.ascontiguousarray(out_tok[L_TXT:])[None, :, :]
    return (img_out.astype(np.float32), txt_out.astype(np.float32))
